# revision 25
# baseline (speedup 1.0000x reference)
"""Trainium2 Bass kernel for nn_AmpLoss_87754771792112.

Strategy: pure data-parallel across 8 NeuronCores. Each core processes a
contiguous 1/8 batch shard and emits per-partition partial sums:
  - Sq1 = sum |dbp-d| /(dbp+d)   (smape halves, factor 2 applied on host)
  - Sq2 = sum |sbp-s| /(sbp+s)
  - masked smape sums and counts for the masks:
      P1  = normal          = (s<120)&(d<80)        == max(s-120, d-80) < 0
      P2  = normal|elevated = (s<130)&(d<80)        == max(s-130, d-80) < 0
      h1  = hyper1          = (130<=s<140)|(80<=d<90) minus (normal|elev)
                            == min(|s-135|, |d-85|) < 5   (exclusion automatic)
      cr  = crisis          = (s>180)|(d>120)       == max(s-180, d-120) > 0
  hyper2 stats are recovered on the host by subtraction (the four
  non-crisis masks partition the space).

The tiny 5-mask sequential accumulation runs on the host in float64
during the gather step.

Only 4 of the 6 inputs are read (m / mbp_pred are dead in the loss).
"""

import numpy as np

try:
    import concourse.bass as bass
except ImportError:  # grading container path
    import sys

    sys.path.insert(0, "/opt/trn_rl_repo")
    import concourse.bass as bass

from contextlib import ExitStack

import concourse.tile as tile
from concourse import mybir
from concourse.bass_utils import run_bass_kernel_spmd

F32 = mybir.dt.float32
ALU = mybir.AluOpType
ACTF = mybir.ActivationFunctionType

B = 16777216
NCORES = 8
PER = B // NCORES  # 2097152
P = 128
F_TILE = 512
N_TILES = PER // (P * F_TILE)  # 16
N_COLS = 10  # accumulator columns per tile iteration

TRACE = False  # set True from test.py for neuron-profile timing
LAST_RESULT = {}

_NC_CACHE = {}


def _split_excess_waits(nc, max_waits=1):
    """This walrus build rejects >1 sync wait on one instruction. Spill the
    excess onto Drain instructions inserted just before, on the same engine."""
    for fn in nc.m.functions:
        for blk in fn.blocks:
            out = []
            for inst in blk.instructions:
                si = inst.sync_info
                if si is not None and si.on_wait and len(si.on_wait) > max_waits:
                    waits = list(si.on_wait)
                    keep, spill = waits[:max_waits], waits[max_waits:]
                    k = 0
                    while spill:
                        chunk, spill = spill[:max_waits], spill[max_waits:]
                        nop = mybir.InstDrain(
                            name=f"{inst.name}-w{k}", engine=inst.engine
                        )
                        nop.sync_info = mybir.SyncInfo(on_wait=chunk, on_update=[])
                        out.append(nop)
                        k += 1
                    inst.sync_info = mybir.SyncInfo(
                        on_wait=keep, on_update=list(si.on_update or [])
                    )
                out.append(inst)
            blk.instructions = out
    return nc


def _build_nc(per_n=PER, f_tile=F_TILE, split=True):
    """Build the single-core Bass graph (same graph runs SPMD on all cores)."""
    n_tiles = per_n // (P * f_tile)
    assert n_tiles * P * f_tile == per_n

    nc = bass.Bass()
    s_e = nc.declare_dram_parameter("s", [per_n], F32, isOutput=False)
    d_e = nc.declare_dram_parameter("d", [per_n], F32, isOutput=False)
    dbp_e = nc.declare_dram_parameter("dbp", [per_n], F32, isOutput=False)
    sbp_e = nc.declare_dram_parameter("sbp", [per_n], F32, isOutput=False)
    out_e = nc.declare_dram_parameter("out", [P, N_COLS * n_tiles], F32, isOutput=True)

    s_r = s_e.rearrange("(t p f) -> t p f", p=P, f=f_tile)
    d_r = d_e.rearrange("(t p f) -> t p f", p=P, f=f_tile)
    dbp_r = dbp_e.rearrange("(t p f) -> t p f", p=P, f=f_tile)
    sbp_r = sbp_e.rearrange("(t p f) -> t p f", p=P, f=f_tile)

    with ExitStack() as ctx:
        tc = ctx.enter_context(tile.TileContext(nc))
        inp = ctx.enter_context(tc.tile_pool(name="inp", bufs=2))
        tmp = ctx.enter_context(tc.tile_pool(name="tmp", bufs=2))
        scr = ctx.enter_context(tc.tile_pool(name="scr", bufs=4))
        accp = ctx.enter_context(tc.tile_pool(name="acc", bufs=1))

        acc = accp.tile([P, N_COLS * n_tiles], F32)

        for t in range(n_tiles):
            base = N_COLS * t

            s_t = inp.tile([P, f_tile], F32, tag="s")
            nc.gpsimd.dma_start(s_t[:], s_r[t])
            d_t = inp.tile([P, f_tile], F32, tag="d")
            nc.gpsimd.dma_start(d_t[:], d_r[t])
            dbp_t = inp.tile([P, f_tile], F32, tag="dbp")
            nc.gpsimd.dma_start(dbp_t[:], dbp_r[t])
            sbp_t = inp.tile([P, f_tile], F32, tag="sbp")
            nc.gpsimd.dma_start(sbp_t[:], sbp_r[t])

            # ---- smape halves: q = |pred - tgt| * 1/(pred + tgt) ----
            t1 = tmp.tile([P, f_tile], F32, tag="t1")
            nc.gpsimd.tensor_sub(t1[:], dbp_t[:], d_t[:])
            den1 = tmp.tile([P, f_tile], F32, tag="den1")
            nc.vector.tensor_add(den1[:], dbp_t[:], d_t[:])
            t2 = tmp.tile([P, f_tile], F32, tag="t2")
            nc.vector.tensor_sub(t2[:], sbp_t[:], s_t[:])
            den2 = tmp.tile([P, f_tile], F32, tag="den2")
            nc.gpsimd.tensor_add(den2[:], sbp_t[:], s_t[:])

            r1 = tmp.tile([P, f_tile], F32, tag="r1")
            nc.vector.reciprocal_approx_fast(out=r1[:], in_=den1[:])
            r2 = tmp.tile([P, f_tile], F32, tag="r2")
            nc.vector.reciprocal_approx_fast(out=r2[:], in_=den2[:])

            q1 = tmp.tile([P, f_tile], F32, tag="q1")
            nc.vector.scalar_tensor_tensor(
                q1[:], t1[:], 0.0, r1[:], op0=ALU.abs_max, op1=ALU.mult,
                accum_out=acc[:, base + 0 : base + 1],
            )
            q2 = tmp.tile([P, f_tile], F32, tag="q2")
            nc.vector.scalar_tensor_tensor(
                q2[:], t2[:], 0.0, r2[:], op0=ALU.abs_max, op1=ALU.mult,
                accum_out=acc[:, base + 1 : base + 2],
            )
            pe = tmp.tile([P, f_tile], F32, tag="pe")
            nc.vector.tensor_add(pe[:], q1[:], q2[:])

            # ---- mask margin tiles ----
            sd80 = tmp.tile([P, f_tile], F32, tag="sd80")
            nc.vector.tensor_scalar(sd80[:], d_t[:], 80.0, None, op0=ALU.subtract)
            d120 = tmp.tile([P, f_tile], F32, tag="d120")
            nc.gpsimd.tensor_scalar(d120[:], d_t[:], 120.0, None, op0=ALU.subtract)
            u1 = tmp.tile([P, f_tile], F32, tag="u1")
            nc.vector.tensor_scalar(u1[:], s_t[:], 135.0, 0.0, op0=ALU.subtract, op1=ALU.abs_max)
            u2 = tmp.tile([P, f_tile], F32, tag="u2")
            nc.gpsimd.tensor_scalar(u2[:], d_t[:], 85.0, 0.0, op0=ALU.subtract, op1=ALU.abs_max)

            m1 = tmp.tile([P, f_tile], F32, tag="m1")
            nc.gpsimd.scalar_tensor_tensor(
                m1[:], s_t[:], 120.0, sd80[:], op0=ALU.subtract, op1=ALU.max
            )
            m2 = tmp.tile([P, f_tile], F32, tag="m2")
            nc.vector.scalar_tensor_tensor(
                m2[:], s_t[:], 130.0, sd80[:], op0=ALU.subtract, op1=ALU.max
            )
            m3 = tmp.tile([P, f_tile], F32, tag="m3")
            nc.gpsimd.tensor_tensor(m3[:], u1[:], u2[:], op=ALU.min)
            m4 = tmp.tile([P, f_tile], F32, tag="m4")
            nc.vector.scalar_tensor_tensor(
                m4[:], s_t[:], 180.0, d120[:], op0=ALU.subtract, op1=ALU.max
            )

            # ---- masked sums (indicator * pe, fused accumulate) ----
            for ci, (m, thr, op) in enumerate(
                [
                    (m1, 0.0, ALU.is_lt),
                    (m2, 0.0, ALU.is_lt),
                    (m3, 5.0, ALU.is_lt),
                    (m4, 0.0, ALU.is_gt),
                ]
            ):
                o = scr.tile([P, f_tile], F32, tag="scr")
                eng = nc.vector if ci % 2 == 0 else nc.gpsimd
                eng.scalar_tensor_tensor(
                    o[:], m[:], thr, pe[:], op0=op, op1=ALU.mult,
                    accum_out=acc[:, base + 2 + ci : base + 3 + ci],
                )

            # ---- mask counts (indicator, fused accumulate) ----
            for ci, (m, thr, op) in enumerate(
                [
                    (m1, 0.0, ALU.is_lt),
                    (m2, 0.0, ALU.is_lt),
                    (m3, 5.0, ALU.is_lt),
                    (m4, 0.0, ALU.is_gt),
                ]
            ):
                o = scr.tile([P, f_tile], F32, tag="scr")
                eng = nc.gpsimd if ci % 2 == 0 else nc.vector
                eng.scalar_tensor_tensor(
                    o[:], m[:], thr, m[:], op0=op, op1=ALU.bypass,
                    accum_out=acc[:, base + 6 + ci : base + 7 + ci],
                )

        nc.gpsimd.dma_start(out_e[:], acc[:])

    return _split_excess_waits(nc) if split else nc


def _build_nc_v2(per_n=PER, f_tile=1024, split=True):
    """v2r: compare-path indicators (bf16) + TensorE diagonal-matmul for the
    masked sums and counts. pe is stored in 130-column blocks (128 data cols +
    a ones column + pad) so one matmul per (mask, block) yields both the
    masked-sum diagonal and the count column. Reciprocal + |.| with fused
    sum accumulation run on the Scalar engine."""
    n_tiles = per_n // (P * f_tile)
    n_blk = f_tile // P
    blkw = 130  # 128 data + 1 ones + 1 pad (4B alignment for bf16 2x mode)
    assert n_tiles * P * f_tile == per_n

    BF16 = mybir.dt.bfloat16
    nc = bass.Bass()
    s_e = nc.declare_dram_parameter("s", [per_n], F32, isOutput=False)
    d_e = nc.declare_dram_parameter("d", [per_n], F32, isOutput=False)
    dbp_e = nc.declare_dram_parameter("dbp", [per_n], F32, isOutput=False)
    sbp_e = nc.declare_dram_parameter("sbp", [per_n], F32, isOutput=False)
    out_q = nc.declare_dram_parameter("out", [P, 2 * n_tiles], F32, isOutput=True)
    out_d = nc.declare_dram_parameter("outd", [4, P, 129], F32, isOutput=True)

    s_r = s_e.rearrange("(t p f) -> t p f", p=P, f=f_tile)
    d_r = d_e.rearrange("(t p f) -> t p f", p=P, f=f_tile)
    dbp_r = dbp_e.rearrange("(t p f) -> t p f", p=P, f=f_tile)
    sbp_r = sbp_e.rearrange("(t p f) -> t p f", p=P, f=f_tile)

    with ExitStack() as ctx:
        tc = ctx.enter_context(tile.TileContext(nc))
        inp = ctx.enter_context(tc.tile_pool(name="inp", bufs=3))
        tmpe = ctx.enter_context(tc.tile_pool(name="tmpe", bufs=3))
        tmp = ctx.enter_context(tc.tile_pool(name="tmp", bufs=2))
        accp = ctx.enter_context(tc.tile_pool(name="acc", bufs=1))
        psum = ctx.enter_context(tc.tile_pool(name="psum", bufs=1, space="PSUM"))

        diag = [
            psum.tile([P, blkw], F32, tag=f"diag{i}", name=f"diag{i}")
            for i in range(4)
        ]
        acc = accp.tile([P, 2 * n_tiles], F32)
        bias_t = accp.tile([P, 2], F32)
        nc.gpsimd.memset(bias_t[:, 0:1], -135.0)
        nc.gpsimd.memset(bias_t[:, 1:2], -85.0)

        def act_recip(out_ap, in_ap):
            nc.scalar.add_instruction(
                mybir.InstActivation(
                    name=nc.get_next_instruction_name(),
                    func=ACTF.Reciprocal,
                    ins=[
                        nc.scalar.lower_ap(in_ap),
                        mybir.ImmediateValue(dtype=F32, value=0.0),
                        mybir.ImmediateValue(dtype=F32, value=1.0),
                        mybir.ImmediateValue(dtype=F32, value=0.0),
                    ],
                    outs=[nc.scalar.lower_ap(out_ap)],
                )
            )

        for t in range(n_tiles):
            s_t = inp.tile([P, f_tile], F32, tag="s")
            nc.gpsimd.dma_start(s_t[:], s_r[t])
            d_t = inp.tile([P, f_tile], F32, tag="d")
            nc.gpsimd.dma_start(d_t[:], d_r[t])
            dbp_t = inp.tile([P, f_tile], F32, tag="dbp")
            nc.gpsimd.dma_start(dbp_t[:], dbp_r[t])
            sbp_t = inp.tile([P, f_tile], F32, tag="sbp")
            nc.gpsimd.dma_start(sbp_t[:], sbp_r[t])

            # ---- smape halves: q = |(pred - tgt) * recip(pred + tgt)| ----
            t1 = tmpe.tile([P, f_tile], F32, tag="t1")
            nc.gpsimd.tensor_sub(t1[:], dbp_t[:], d_t[:])
            den1 = tmpe.tile([P, f_tile], F32, tag="den1")
            nc.vector.tensor_add(den1[:], dbp_t[:], d_t[:])
            t2 = tmpe.tile([P, f_tile], F32, tag="t2")
            nc.vector.tensor_sub(t2[:], sbp_t[:], s_t[:])
            den2 = tmpe.tile([P, f_tile], F32, tag="den2")
            nc.gpsimd.tensor_add(den2[:], sbp_t[:], s_t[:])

            r1 = tmp.tile([P, f_tile], F32, tag="r1")
            act_recip(r1[:], den1[:])
            r2 = tmp.tile([P, f_tile], F32, tag="r2")
            act_recip(r2[:], den2[:])

            w1 = tmp.tile([P, f_tile], F32, tag="w1")
            nc.vector.tensor_mul(w1[:], t1[:], r1[:])
            w2 = tmp.tile([P, f_tile], F32, tag="w2")
            nc.vector.tensor_mul(w2[:], t2[:], r2[:])
            aq1 = tmp.tile([P, f_tile], BF16, tag="aq1")
            nc.scalar.activation(
                aq1[:], w1[:], ACTF.Abs, accum_out=acc[:, 2 * t : 2 * t + 1]
            )
            aq2 = tmp.tile([P, f_tile], BF16, tag="aq2")
            nc.scalar.activation(
                aq2[:], w2[:], ACTF.Abs, accum_out=acc[:, 2 * t + 1 : 2 * t + 2]
            )

            # pe in 130-col blocks: cols k*130..k*130+127 data, k*130+128 ones
            pe = tmp.tile([P, n_blk * blkw], BF16, tag="pe")
            pe3 = pe[:].rearrange("p (b w) -> p b w", w=blkw)
            nc.vector.memset(pe3[:, :, 128:130], 1.0)
            nc.vector.tensor_add(pe3[:, :, 0:128], aq1[:].rearrange("p (b w) -> p b w", w=P), aq2[:].rearrange("p (b w) -> p b w", w=P))

            # ---- indicators (bf16, sign-exact) ----
            c1 = tmp.tile([P, f_tile], BF16, tag="c1")
            nc.vector.tensor_scalar(c1[:], s_t[:], 120.0, None, op0=ALU.is_lt)
            c2 = tmp.tile([P, f_tile], BF16, tag="c2")
            nc.vector.tensor_scalar(c2[:], s_t[:], 130.0, None, op0=ALU.is_lt)
            g1 = tmp.tile([P, f_tile], BF16, tag="g1")
            nc.vector.tensor_scalar(g1[:], d_t[:], 80.0, None, op0=ALU.is_lt)
            u1 = tmp.tile([P, f_tile], F32, tag="u1")
            nc.scalar.activation(u1[:], s_t[:], ACTF.Abs, bias=bias_t[:, 0:1])
            u2 = tmp.tile([P, f_tile], F32, tag="u2")
            nc.scalar.activation(u2[:], d_t[:], ACTF.Abs, bias=bias_t[:, 1:2])
            c4 = tmp.tile([P, f_tile], BF16, tag="c4")
            nc.vector.tensor_scalar(c4[:], s_t[:], 180.0, None, op0=ALU.is_gt)
            g3 = tmp.tile([P, f_tile], BF16, tag="g3")
            nc.vector.tensor_scalar(g3[:], d_t[:], 120.0, None, op0=ALU.is_gt)

            p1m = tmp.tile([P, f_tile], BF16, tag="p1m")
            nc.gpsimd.tensor_mul(p1m[:], c1[:], g1[:])
            p2m = tmp.tile([P, f_tile], BF16, tag="p2m")
            nc.vector.tensor_mul(p2m[:], c2[:], g1[:])
            m3 = tmp.tile([P, f_tile], F32, tag="m3")
            nc.vector.tensor_tensor(m3[:], u1[:], u2[:], op=ALU.min)
            h1m = tmp.tile([P, f_tile], BF16, tag="h1m")
            nc.vector.tensor_scalar(h1m[:], m3[:], 5.0, None, op0=ALU.is_lt)
            crm = tmp.tile([P, f_tile], BF16, tag="crm")
            nc.vector.tensor_tensor(crm[:], c4[:], g3[:], op=ALU.max)

            for mi, mk in enumerate([p1m, p2m, h1m, crm]):
                for blk in range(n_blk):
                    first = t == 0 and blk == 0
                    last = t == n_tiles - 1 and blk == n_blk - 1
                    lo = blk * P
                    nc.tensor.matmul(
                        diag[mi][:, 0:129],
                        mk[:, lo : lo + P],
                        pe[:, blk * blkw : blk * blkw + 129],
                        start=first,
                        stop=last,
                    )

        stage_d = accp.tile([P, 4 * 129], F32)
        for i in range(4):
            nc.vector.tensor_copy(stage_d[:, i * 129 : (i + 1) * 129], diag[i][:, 0:129])
            nc.gpsimd.dma_start(out_d[i], stage_d[:, i * 129 : (i + 1) * 129])
        nc.gpsimd.dma_start(out_q[:], acc[:])

    return _split_excess_waits(nc) if split else nc


VERSION = 2


def _get_nc(per_n=PER, f_tile=None):
    if f_tile is None:
        f_tile = 1024 if VERSION == 2 else F_TILE
    key = (VERSION, per_n, f_tile)
    if key not in _NC_CACHE:
        builder = _build_nc_v2 if VERSION == 2 else _build_nc
        _NC_CACHE[key] = builder(per_n, f_tile)
    return _NC_CACHE[key]


def _finalize(vec10, batch_n):
    """Host-side: combine the 10 global partial sums into the loss (f64)."""
    sq1, sq2, s_p1, s_p2, s_h1, s_cr, c_p1, c_p2, c_h1, c_cr = [
        float(x) for x in vec10
    ]
    s_tot = sq1 + sq2
    # factor 2 from smape definition
    S = [
        2.0 * s_p1,                       # normal
        2.0 * (s_p2 - s_p1),              # elevated
        2.0 * s_h1,                       # hyper1
        2.0 * (s_tot - s_p2 - s_h1),      # hyper2
        2.0 * s_cr,                       # crisis
    ]
    C = [
        c_p1,
        c_p2 - c_p1,
        c_h1,
        batch_n - c_p2 - c_h1,
        c_cr,
    ]
    rst = 0.0
    m_rst = 0.0
    mask_cnt = 0
    for s_m, cnt in zip(S, C):
        w = np.sqrt(np.log(batch_n / max(cnt, 1.0)))
        if cnt > 0:
            m_rst = (m_rst + s_m * w) / cnt / 2.0
            rst = rst + m_rst
            mask_cnt += 1
    if mask_cnt == 0:
        return rst / 5.0
    return rst / mask_cnt


def host_partials(s, d, dbp, sbp):
    """Numpy replica of the device partials (for testing)."""
    s = s.astype(np.float64)
    d = d.astype(np.float64)
    dbp = dbp.astype(np.float64)
    sbp = sbp.astype(np.float64)
    q1 = np.abs(dbp - d) / (dbp + d)
    q2 = np.abs(sbp - s) / (sbp + s)
    pe = q1 + q2
    m1 = np.maximum(s - 120, d - 80)
    m2 = np.maximum(s - 130, d - 80)
    m3 = np.minimum(np.abs(s - 135), np.abs(d - 85))
    m4 = np.maximum(s - 180, d - 120)
    return np.array(
        [
            q1.sum(),
            q2.sum(),
            pe[m1 < 0].sum(),
            pe[m2 < 0].sum(),
            pe[m3 < 5].sum(),
            pe[m4 > 0].sum(),
            (m1 < 0).sum(),
            (m2 < 0).sum(),
            (m3 < 5).sum(),
            (m4 > 0).sum(),
        ]
    )


def kernel(**inputs):
    s = np.ascontiguousarray(np.asarray(inputs["s"], dtype=np.float32).reshape(-1))
    d = np.ascontiguousarray(np.asarray(inputs["d"], dtype=np.float32).reshape(-1))
    dbp = np.ascontiguousarray(
        np.asarray(inputs["dbp_pred"], dtype=np.float32).reshape(-1)
    )
    sbp = np.ascontiguousarray(
        np.asarray(inputs["sbp_pred"], dtype=np.float32).reshape(-1)
    )
    batch_n = s.shape[0]
    assert batch_n == B, f"expected {B}, got {batch_n}"

    nc = _get_nc()
    in_maps = []
    for c in range(NCORES):
        sl = slice(c * PER, (c + 1) * PER)
        in_maps.append({"s": s[sl], "d": d[sl], "dbp": dbp[sl], "sbp": sbp[sl]})

    res = run_bass_kernel_spmd(nc, in_maps, list(range(NCORES)), trace=TRACE)
    LAST_RESULT["exec_time_ns"] = res.exec_time_ns
    LAST_RESULT["raw"] = res

    tot = np.zeros(N_COLS, np.float64)
    for r in res.results:
        if VERSION == 2:
            q = np.asarray(r["out"], np.float64)  # [P, 2*n_tiles]
            diag = np.asarray(r["outd"], np.float64)  # [4, P, 130]
            tot[0] += q[:, 0::2].sum()
            tot[1] += q[:, 1::2].sum()
            for i in range(4):
                tot[2 + i] += np.trace(diag[i, :, 0:128])
                tot[6 + i] += diag[i, :, 128].sum()
        else:
            o = np.asarray(r["out"], np.float64).reshape(P, N_TILES, N_COLS)
            tot += o.sum(axis=(0, 1))
    loss = _finalize(tot, float(batch_n))
    return np.float32(loss)


# revision 26
# speedup vs baseline: 1.0327x; 1.0327x over previous
"""Trainium2 Bass kernel for nn_AmpLoss_87754771792112.

Strategy: pure data-parallel across 8 NeuronCores. Each core processes a
contiguous 1/8 batch shard and emits per-partition partial sums:
  - Sq1 = sum |dbp-d| /(dbp+d)   (smape halves, factor 2 applied on host)
  - Sq2 = sum |sbp-s| /(sbp+s)
  - masked smape sums and counts for the masks:
      P1  = normal          = (s<120)&(d<80)        == max(s-120, d-80) < 0
      P2  = normal|elevated = (s<130)&(d<80)        == max(s-130, d-80) < 0
      h1  = hyper1          = (130<=s<140)|(80<=d<90) minus (normal|elev)
                            == min(|s-135|, |d-85|) < 5   (exclusion automatic)
      cr  = crisis          = (s>180)|(d>120)       == max(s-180, d-120) > 0
  hyper2 stats are recovered on the host by subtraction (the four
  non-crisis masks partition the space).

The tiny 5-mask sequential accumulation runs on the host in float64
during the gather step.

Only 4 of the 6 inputs are read (m / mbp_pred are dead in the loss).
"""

import numpy as np

try:
    import concourse.bass as bass
except ImportError:  # grading container path
    import sys

    sys.path.insert(0, "/opt/trn_rl_repo")
    import concourse.bass as bass

from contextlib import ExitStack

import concourse.tile as tile
from concourse import mybir
from concourse.bass_utils import run_bass_kernel_spmd

F32 = mybir.dt.float32
ALU = mybir.AluOpType
ACTF = mybir.ActivationFunctionType

B = 16777216
NCORES = 8
PER = B // NCORES  # 2097152
P = 128
F_TILE = 512
N_TILES = PER // (P * F_TILE)  # 16
N_COLS = 10  # accumulator columns per tile iteration

TRACE = False  # set True from test.py for neuron-profile timing
LAST_RESULT = {}

_NC_CACHE = {}


def _split_excess_waits(nc, max_waits=1):
    """This walrus build rejects >1 sync wait on one instruction. Spill the
    excess onto Drain instructions inserted just before, on the same engine."""
    for fn in nc.m.functions:
        for blk in fn.blocks:
            out = []
            for inst in blk.instructions:
                si = inst.sync_info
                if si is not None and si.on_wait and len(si.on_wait) > max_waits:
                    waits = list(si.on_wait)
                    keep, spill = waits[:max_waits], waits[max_waits:]
                    k = 0
                    while spill:
                        chunk, spill = spill[:max_waits], spill[max_waits:]
                        nop = mybir.InstDrain(
                            name=f"{inst.name}-w{k}", engine=inst.engine
                        )
                        nop.sync_info = mybir.SyncInfo(on_wait=chunk, on_update=[])
                        out.append(nop)
                        k += 1
                    inst.sync_info = mybir.SyncInfo(
                        on_wait=keep, on_update=list(si.on_update or [])
                    )
                out.append(inst)
            blk.instructions = out
    return nc


def _build_nc(per_n=PER, f_tile=F_TILE, split=True):
    """Build the single-core Bass graph (same graph runs SPMD on all cores)."""
    n_tiles = per_n // (P * f_tile)
    assert n_tiles * P * f_tile == per_n

    nc = bass.Bass()
    s_e = nc.declare_dram_parameter("s", [per_n], F32, isOutput=False)
    d_e = nc.declare_dram_parameter("d", [per_n], F32, isOutput=False)
    dbp_e = nc.declare_dram_parameter("dbp", [per_n], F32, isOutput=False)
    sbp_e = nc.declare_dram_parameter("sbp", [per_n], F32, isOutput=False)
    out_e = nc.declare_dram_parameter("out", [P, N_COLS * n_tiles], F32, isOutput=True)

    s_r = s_e.rearrange("(t p f) -> t p f", p=P, f=f_tile)
    d_r = d_e.rearrange("(t p f) -> t p f", p=P, f=f_tile)
    dbp_r = dbp_e.rearrange("(t p f) -> t p f", p=P, f=f_tile)
    sbp_r = sbp_e.rearrange("(t p f) -> t p f", p=P, f=f_tile)

    with ExitStack() as ctx:
        tc = ctx.enter_context(tile.TileContext(nc))
        inp = ctx.enter_context(tc.tile_pool(name="inp", bufs=2))
        tmp = ctx.enter_context(tc.tile_pool(name="tmp", bufs=2))
        scr = ctx.enter_context(tc.tile_pool(name="scr", bufs=4))
        accp = ctx.enter_context(tc.tile_pool(name="acc", bufs=1))

        acc = accp.tile([P, N_COLS * n_tiles], F32)

        for t in range(n_tiles):
            base = N_COLS * t

            s_t = inp.tile([P, f_tile], F32, tag="s")
            nc.gpsimd.dma_start(s_t[:], s_r[t])
            d_t = inp.tile([P, f_tile], F32, tag="d")
            nc.gpsimd.dma_start(d_t[:], d_r[t])
            dbp_t = inp.tile([P, f_tile], F32, tag="dbp")
            nc.gpsimd.dma_start(dbp_t[:], dbp_r[t])
            sbp_t = inp.tile([P, f_tile], F32, tag="sbp")
            nc.gpsimd.dma_start(sbp_t[:], sbp_r[t])

            # ---- smape halves: q = |pred - tgt| * 1/(pred + tgt) ----
            t1 = tmp.tile([P, f_tile], F32, tag="t1")
            nc.gpsimd.tensor_sub(t1[:], dbp_t[:], d_t[:])
            den1 = tmp.tile([P, f_tile], F32, tag="den1")
            nc.vector.tensor_add(den1[:], dbp_t[:], d_t[:])
            t2 = tmp.tile([P, f_tile], F32, tag="t2")
            nc.vector.tensor_sub(t2[:], sbp_t[:], s_t[:])
            den2 = tmp.tile([P, f_tile], F32, tag="den2")
            nc.gpsimd.tensor_add(den2[:], sbp_t[:], s_t[:])

            r1 = tmp.tile([P, f_tile], F32, tag="r1")
            nc.vector.reciprocal_approx_fast(out=r1[:], in_=den1[:])
            r2 = tmp.tile([P, f_tile], F32, tag="r2")
            nc.vector.reciprocal_approx_fast(out=r2[:], in_=den2[:])

            q1 = tmp.tile([P, f_tile], F32, tag="q1")
            nc.vector.scalar_tensor_tensor(
                q1[:], t1[:], 0.0, r1[:], op0=ALU.abs_max, op1=ALU.mult,
                accum_out=acc[:, base + 0 : base + 1],
            )
            q2 = tmp.tile([P, f_tile], F32, tag="q2")
            nc.vector.scalar_tensor_tensor(
                q2[:], t2[:], 0.0, r2[:], op0=ALU.abs_max, op1=ALU.mult,
                accum_out=acc[:, base + 1 : base + 2],
            )
            pe = tmp.tile([P, f_tile], F32, tag="pe")
            nc.vector.tensor_add(pe[:], q1[:], q2[:])

            # ---- mask margin tiles ----
            sd80 = tmp.tile([P, f_tile], F32, tag="sd80")
            nc.vector.tensor_scalar(sd80[:], d_t[:], 80.0, None, op0=ALU.subtract)
            d120 = tmp.tile([P, f_tile], F32, tag="d120")
            nc.gpsimd.tensor_scalar(d120[:], d_t[:], 120.0, None, op0=ALU.subtract)
            u1 = tmp.tile([P, f_tile], F32, tag="u1")
            nc.vector.tensor_scalar(u1[:], s_t[:], 135.0, 0.0, op0=ALU.subtract, op1=ALU.abs_max)
            u2 = tmp.tile([P, f_tile], F32, tag="u2")
            nc.gpsimd.tensor_scalar(u2[:], d_t[:], 85.0, 0.0, op0=ALU.subtract, op1=ALU.abs_max)

            m1 = tmp.tile([P, f_tile], F32, tag="m1")
            nc.gpsimd.scalar_tensor_tensor(
                m1[:], s_t[:], 120.0, sd80[:], op0=ALU.subtract, op1=ALU.max
            )
            m2 = tmp.tile([P, f_tile], F32, tag="m2")
            nc.vector.scalar_tensor_tensor(
                m2[:], s_t[:], 130.0, sd80[:], op0=ALU.subtract, op1=ALU.max
            )
            m3 = tmp.tile([P, f_tile], F32, tag="m3")
            nc.gpsimd.tensor_tensor(m3[:], u1[:], u2[:], op=ALU.min)
            m4 = tmp.tile([P, f_tile], F32, tag="m4")
            nc.vector.scalar_tensor_tensor(
                m4[:], s_t[:], 180.0, d120[:], op0=ALU.subtract, op1=ALU.max
            )

            # ---- masked sums (indicator * pe, fused accumulate) ----
            for ci, (m, thr, op) in enumerate(
                [
                    (m1, 0.0, ALU.is_lt),
                    (m2, 0.0, ALU.is_lt),
                    (m3, 5.0, ALU.is_lt),
                    (m4, 0.0, ALU.is_gt),
                ]
            ):
                o = scr.tile([P, f_tile], F32, tag="scr")
                eng = nc.vector if ci % 2 == 0 else nc.gpsimd
                eng.scalar_tensor_tensor(
                    o[:], m[:], thr, pe[:], op0=op, op1=ALU.mult,
                    accum_out=acc[:, base + 2 + ci : base + 3 + ci],
                )

            # ---- mask counts (indicator, fused accumulate) ----
            for ci, (m, thr, op) in enumerate(
                [
                    (m1, 0.0, ALU.is_lt),
                    (m2, 0.0, ALU.is_lt),
                    (m3, 5.0, ALU.is_lt),
                    (m4, 0.0, ALU.is_gt),
                ]
            ):
                o = scr.tile([P, f_tile], F32, tag="scr")
                eng = nc.gpsimd if ci % 2 == 0 else nc.vector
                eng.scalar_tensor_tensor(
                    o[:], m[:], thr, m[:], op0=op, op1=ALU.bypass,
                    accum_out=acc[:, base + 6 + ci : base + 7 + ci],
                )

        nc.gpsimd.dma_start(out_e[:], acc[:])

    return _split_excess_waits(nc) if split else nc


def _build_nc_v2(per_n=PER, f_tile=1024, split=True):
    """v2r: compare-path indicators (bf16) + TensorE diagonal-matmul for the
    masked sums and counts. pe is stored in 130-column blocks (128 data cols +
    a ones column + pad) so one matmul per (mask, block) yields both the
    masked-sum diagonal and the count column. Reciprocal + |.| with fused
    sum accumulation run on the Scalar engine."""
    n_tiles = per_n // (P * f_tile)
    n_blk = f_tile // P
    blkw = 130  # 128 data + 1 ones + 1 pad (4B alignment for bf16 2x mode)
    assert n_tiles * P * f_tile == per_n

    BF16 = mybir.dt.bfloat16
    nc = bass.Bass()
    s_e = nc.declare_dram_parameter("s", [per_n], F32, isOutput=False)
    d_e = nc.declare_dram_parameter("d", [per_n], F32, isOutput=False)
    dbp_e = nc.declare_dram_parameter("dbp", [per_n], F32, isOutput=False)
    sbp_e = nc.declare_dram_parameter("sbp", [per_n], F32, isOutput=False)
    out_q = nc.declare_dram_parameter("out", [P, 2 * n_tiles], F32, isOutput=True)
    out_d = nc.declare_dram_parameter("outd", [4, P, 129], F32, isOutput=True)

    s_r = s_e.rearrange("(t p f) -> t p f", p=P, f=f_tile)
    d_r = d_e.rearrange("(t p f) -> t p f", p=P, f=f_tile)
    dbp_r = dbp_e.rearrange("(t p f) -> t p f", p=P, f=f_tile)
    sbp_r = sbp_e.rearrange("(t p f) -> t p f", p=P, f=f_tile)

    with ExitStack() as ctx:
        tc = ctx.enter_context(tile.TileContext(nc))
        inp = ctx.enter_context(tc.tile_pool(name="inp", bufs=3))
        tmp = ctx.enter_context(tc.tile_pool(name="tmp", bufs=2))
        accp = ctx.enter_context(tc.tile_pool(name="acc", bufs=1))
        psum = ctx.enter_context(tc.tile_pool(name="psum", bufs=1, space="PSUM"))

        diag = [
            psum.tile([P, blkw], F32, tag=f"diag{i}", name=f"diag{i}")
            for i in range(4)
        ]
        acc = accp.tile([P, 2 * n_tiles], F32)
        bias_t = accp.tile([P, 2], F32)
        nc.gpsimd.memset(bias_t[:, 0:1], -135.0)
        nc.gpsimd.memset(bias_t[:, 1:2], -85.0)

        def act_recip(out_ap, in_ap):
            nc.scalar.add_instruction(
                mybir.InstActivation(
                    name=nc.get_next_instruction_name(),
                    func=ACTF.Reciprocal,
                    ins=[
                        nc.scalar.lower_ap(in_ap),
                        mybir.ImmediateValue(dtype=F32, value=0.0),
                        mybir.ImmediateValue(dtype=F32, value=1.0),
                        mybir.ImmediateValue(dtype=F32, value=0.0),
                    ],
                    outs=[nc.scalar.lower_ap(out_ap)],
                )
            )

        for t in range(n_tiles):
            s_t = inp.tile([P, f_tile], F32, tag="s")
            nc.gpsimd.dma_start(s_t[:], s_r[t])
            d_t = inp.tile([P, f_tile], F32, tag="d")
            nc.gpsimd.dma_start(d_t[:], d_r[t])
            dbp_t = inp.tile([P, f_tile], F32, tag="dbp")
            nc.gpsimd.dma_start(dbp_t[:], dbp_r[t])
            sbp_t = inp.tile([P, f_tile], F32, tag="sbp")
            nc.gpsimd.dma_start(sbp_t[:], sbp_r[t])

            # ---- smape halves: q = |(pred - tgt) * recip(pred + tgt)| ----
            t1 = tmp.tile([P, f_tile], F32, tag="t1")
            nc.gpsimd.tensor_sub(t1[:], dbp_t[:], d_t[:])
            den1 = tmp.tile([P, f_tile], F32, tag="den1")
            nc.vector.tensor_add(den1[:], dbp_t[:], d_t[:])
            t2 = tmp.tile([P, f_tile], F32, tag="t2")
            nc.vector.tensor_sub(t2[:], sbp_t[:], s_t[:])
            den2 = tmp.tile([P, f_tile], F32, tag="den2")
            nc.gpsimd.tensor_add(den2[:], sbp_t[:], s_t[:])

            r1 = tmp.tile([P, f_tile], F32, tag="r1")
            act_recip(r1[:], den1[:])
            r2 = tmp.tile([P, f_tile], F32, tag="r2")
            act_recip(r2[:], den2[:])

            w1 = tmp.tile([P, f_tile], F32, tag="w1")
            nc.vector.tensor_mul(w1[:], t1[:], r1[:])
            w2 = tmp.tile([P, f_tile], F32, tag="w2")
            nc.vector.tensor_mul(w2[:], t2[:], r2[:])
            aq1 = tmp.tile([P, f_tile], BF16, tag="aq1")
            nc.scalar.activation(
                aq1[:], w1[:], ACTF.Abs, accum_out=acc[:, 2 * t : 2 * t + 1]
            )
            aq2 = tmp.tile([P, f_tile], BF16, tag="aq2")
            nc.scalar.activation(
                aq2[:], w2[:], ACTF.Abs, accum_out=acc[:, 2 * t + 1 : 2 * t + 2]
            )

            # pe in 130-col blocks: cols k*130..k*130+127 data, k*130+128 ones
            pe = tmp.tile([P, n_blk * blkw], BF16, tag="pe")
            pe3 = pe[:].rearrange("p (b w) -> p b w", w=blkw)
            nc.vector.memset(pe3[:, :, 128:130], 1.0)
            nc.vector.tensor_add(pe3[:, :, 0:128], aq1[:].rearrange("p (b w) -> p b w", w=P), aq2[:].rearrange("p (b w) -> p b w", w=P))

            # ---- indicators (bf16, sign-exact) ----
            c1 = tmp.tile([P, f_tile], BF16, tag="c1")
            nc.vector.tensor_scalar(c1[:], s_t[:], 120.0, None, op0=ALU.is_lt)
            c2 = tmp.tile([P, f_tile], BF16, tag="c2")
            nc.vector.tensor_scalar(c2[:], s_t[:], 130.0, None, op0=ALU.is_lt)
            g1 = tmp.tile([P, f_tile], BF16, tag="g1")
            nc.vector.tensor_scalar(g1[:], d_t[:], 80.0, None, op0=ALU.is_lt)
            u1 = tmp.tile([P, f_tile], F32, tag="u1")
            nc.scalar.activation(u1[:], s_t[:], ACTF.Abs, bias=bias_t[:, 0:1])
            u2 = tmp.tile([P, f_tile], F32, tag="u2")
            nc.scalar.activation(u2[:], d_t[:], ACTF.Abs, bias=bias_t[:, 1:2])
            c4 = tmp.tile([P, f_tile], BF16, tag="c4")
            nc.vector.tensor_scalar(c4[:], s_t[:], 180.0, None, op0=ALU.is_gt)
            g3 = tmp.tile([P, f_tile], BF16, tag="g3")
            nc.vector.tensor_scalar(g3[:], d_t[:], 120.0, None, op0=ALU.is_gt)

            p1m = tmp.tile([P, f_tile], BF16, tag="p1m")
            nc.gpsimd.tensor_mul(p1m[:], c1[:], g1[:])
            p2m = tmp.tile([P, f_tile], BF16, tag="p2m")
            nc.vector.tensor_mul(p2m[:], c2[:], g1[:])
            m3 = tmp.tile([P, f_tile], F32, tag="m3")
            nc.vector.tensor_tensor(m3[:], u1[:], u2[:], op=ALU.min)
            h1m = tmp.tile([P, f_tile], BF16, tag="h1m")
            nc.vector.tensor_scalar(h1m[:], m3[:], 5.0, None, op0=ALU.is_lt)
            crm = tmp.tile([P, f_tile], BF16, tag="crm")
            nc.vector.tensor_tensor(crm[:], c4[:], g3[:], op=ALU.max)

            for mi, mk in enumerate([p1m, p2m, h1m, crm]):
                for blk in range(n_blk):
                    first = t == 0 and blk == 0
                    last = t == n_tiles - 1 and blk == n_blk - 1
                    lo = blk * P
                    nc.tensor.matmul(
                        diag[mi][:, 0:129],
                        mk[:, lo : lo + P],
                        pe[:, blk * blkw : blk * blkw + 129],
                        start=first,
                        stop=last,
                    )

        stage_d = accp.tile([P, 4 * 129], F32)
        for i in range(4):
            nc.vector.tensor_copy(stage_d[:, i * 129 : (i + 1) * 129], diag[i][:, 0:129])
            nc.gpsimd.dma_start(out_d[i], stage_d[:, i * 129 : (i + 1) * 129])
        nc.gpsimd.dma_start(out_q[:], acc[:])

    return _split_excess_waits(nc) if split else nc


VERSION = 2


def _get_nc(per_n=PER, f_tile=None):
    if f_tile is None:
        f_tile = 1024 if VERSION == 2 else F_TILE
    key = (VERSION, per_n, f_tile)
    if key not in _NC_CACHE:
        builder = _build_nc_v2 if VERSION == 2 else _build_nc
        _NC_CACHE[key] = builder(per_n, f_tile)
    return _NC_CACHE[key]


def _finalize(vec10, batch_n):
    """Host-side: combine the 10 global partial sums into the loss (f64)."""
    sq1, sq2, s_p1, s_p2, s_h1, s_cr, c_p1, c_p2, c_h1, c_cr = [
        float(x) for x in vec10
    ]
    s_tot = sq1 + sq2
    # factor 2 from smape definition
    S = [
        2.0 * s_p1,                       # normal
        2.0 * (s_p2 - s_p1),              # elevated
        2.0 * s_h1,                       # hyper1
        2.0 * (s_tot - s_p2 - s_h1),      # hyper2
        2.0 * s_cr,                       # crisis
    ]
    C = [
        c_p1,
        c_p2 - c_p1,
        c_h1,
        batch_n - c_p2 - c_h1,
        c_cr,
    ]
    rst = 0.0
    m_rst = 0.0
    mask_cnt = 0
    for s_m, cnt in zip(S, C):
        w = np.sqrt(np.log(batch_n / max(cnt, 1.0)))
        if cnt > 0:
            m_rst = (m_rst + s_m * w) / cnt / 2.0
            rst = rst + m_rst
            mask_cnt += 1
    if mask_cnt == 0:
        return rst / 5.0
    return rst / mask_cnt


def host_partials(s, d, dbp, sbp):
    """Numpy replica of the device partials (for testing)."""
    s = s.astype(np.float64)
    d = d.astype(np.float64)
    dbp = dbp.astype(np.float64)
    sbp = sbp.astype(np.float64)
    q1 = np.abs(dbp - d) / (dbp + d)
    q2 = np.abs(sbp - s) / (sbp + s)
    pe = q1 + q2
    m1 = np.maximum(s - 120, d - 80)
    m2 = np.maximum(s - 130, d - 80)
    m3 = np.minimum(np.abs(s - 135), np.abs(d - 85))
    m4 = np.maximum(s - 180, d - 120)
    return np.array(
        [
            q1.sum(),
            q2.sum(),
            pe[m1 < 0].sum(),
            pe[m2 < 0].sum(),
            pe[m3 < 5].sum(),
            pe[m4 > 0].sum(),
            (m1 < 0).sum(),
            (m2 < 0).sum(),
            (m3 < 5).sum(),
            (m4 > 0).sum(),
        ]
    )


def kernel(**inputs):
    s = np.ascontiguousarray(np.asarray(inputs["s"], dtype=np.float32).reshape(-1))
    d = np.ascontiguousarray(np.asarray(inputs["d"], dtype=np.float32).reshape(-1))
    dbp = np.ascontiguousarray(
        np.asarray(inputs["dbp_pred"], dtype=np.float32).reshape(-1)
    )
    sbp = np.ascontiguousarray(
        np.asarray(inputs["sbp_pred"], dtype=np.float32).reshape(-1)
    )
    batch_n = s.shape[0]
    assert batch_n == B, f"expected {B}, got {batch_n}"

    nc = _get_nc()
    in_maps = []
    for c in range(NCORES):
        sl = slice(c * PER, (c + 1) * PER)
        in_maps.append({"s": s[sl], "d": d[sl], "dbp": dbp[sl], "sbp": sbp[sl]})

    res = run_bass_kernel_spmd(nc, in_maps, list(range(NCORES)), trace=TRACE)
    LAST_RESULT["exec_time_ns"] = res.exec_time_ns
    LAST_RESULT["raw"] = res

    tot = np.zeros(N_COLS, np.float64)
    for r in res.results:
        if VERSION == 2:
            q = np.asarray(r["out"], np.float64)  # [P, 2*n_tiles]
            diag = np.asarray(r["outd"], np.float64)  # [4, P, 130]
            tot[0] += q[:, 0::2].sum()
            tot[1] += q[:, 1::2].sum()
            for i in range(4):
                tot[2 + i] += np.trace(diag[i, :, 0:128])
                tot[6 + i] += diag[i, :, 128].sum()
        else:
            o = np.asarray(r["out"], np.float64).reshape(P, N_TILES, N_COLS)
            tot += o.sum(axis=(0, 1))
    loss = _finalize(tot, float(batch_n))
    return np.float32(loss)


# revision 27
# speedup vs baseline: 1.0861x; 1.0517x over previous
"""Trainium2 Bass kernel for nn_AmpLoss_87754771792112.

Strategy: pure data-parallel across 8 NeuronCores. Each core processes a
contiguous 1/8 batch shard and emits per-partition partial sums:
  - Sq1 = sum |dbp-d| /(dbp+d)   (smape halves, factor 2 applied on host)
  - Sq2 = sum |sbp-s| /(sbp+s)
  - masked smape sums and counts for the masks:
      P1  = normal          = (s<120)&(d<80)        == max(s-120, d-80) < 0
      P2  = normal|elevated = (s<130)&(d<80)        == max(s-130, d-80) < 0
      h1  = hyper1          = (130<=s<140)|(80<=d<90) minus (normal|elev)
                            == min(|s-135|, |d-85|) < 5   (exclusion automatic)
      cr  = crisis          = (s>180)|(d>120)       == max(s-180, d-120) > 0
  hyper2 stats are recovered on the host by subtraction (the four
  non-crisis masks partition the space).

The tiny 5-mask sequential accumulation runs on the host in float64
during the gather step.

Only 4 of the 6 inputs are read (m / mbp_pred are dead in the loss).
"""

import numpy as np

try:
    import concourse.bass as bass
except ImportError:  # grading container path
    import sys

    sys.path.insert(0, "/opt/trn_rl_repo")
    import concourse.bass as bass

from contextlib import ExitStack

import concourse.tile as tile
from concourse import mybir
from concourse.bass_utils import run_bass_kernel_spmd

F32 = mybir.dt.float32
ALU = mybir.AluOpType
ACTF = mybir.ActivationFunctionType

B = 16777216
NCORES = 8
PER = B // NCORES  # 2097152
P = 128
F_TILE = 512
N_TILES = PER // (P * F_TILE)  # 16
N_COLS = 10  # accumulator columns per tile iteration

TRACE = False  # set True from test.py for neuron-profile timing
LAST_RESULT = {}

_NC_CACHE = {}


def _split_excess_waits(nc, max_waits=1):
    """This walrus build rejects >1 sync wait on one instruction. Spill the
    excess onto Drain instructions inserted just before, on the same engine."""
    for fn in nc.m.functions:
        for blk in fn.blocks:
            out = []
            for inst in blk.instructions:
                si = inst.sync_info
                if si is not None and si.on_wait and len(si.on_wait) > max_waits:
                    waits = list(si.on_wait)
                    keep, spill = waits[:max_waits], waits[max_waits:]
                    k = 0
                    while spill:
                        chunk, spill = spill[:max_waits], spill[max_waits:]
                        nop = mybir.InstDrain(
                            name=f"{inst.name}-w{k}", engine=inst.engine
                        )
                        nop.sync_info = mybir.SyncInfo(on_wait=chunk, on_update=[])
                        out.append(nop)
                        k += 1
                    inst.sync_info = mybir.SyncInfo(
                        on_wait=keep, on_update=list(si.on_update or [])
                    )
                out.append(inst)
            blk.instructions = out
    return nc


def _build_nc(per_n=PER, f_tile=F_TILE, split=True):
    """Build the single-core Bass graph (same graph runs SPMD on all cores)."""
    n_tiles = per_n // (P * f_tile)
    assert n_tiles * P * f_tile == per_n

    nc = bass.Bass()
    s_e = nc.declare_dram_parameter("s", [per_n], F32, isOutput=False)
    d_e = nc.declare_dram_parameter("d", [per_n], F32, isOutput=False)
    dbp_e = nc.declare_dram_parameter("dbp", [per_n], F32, isOutput=False)
    sbp_e = nc.declare_dram_parameter("sbp", [per_n], F32, isOutput=False)
    out_e = nc.declare_dram_parameter("out", [P, N_COLS * n_tiles], F32, isOutput=True)

    s_r = s_e.rearrange("(t p f) -> t p f", p=P, f=f_tile)
    d_r = d_e.rearrange("(t p f) -> t p f", p=P, f=f_tile)
    dbp_r = dbp_e.rearrange("(t p f) -> t p f", p=P, f=f_tile)
    sbp_r = sbp_e.rearrange("(t p f) -> t p f", p=P, f=f_tile)

    with ExitStack() as ctx:
        tc = ctx.enter_context(tile.TileContext(nc))
        inp = ctx.enter_context(tc.tile_pool(name="inp", bufs=2))
        tmp = ctx.enter_context(tc.tile_pool(name="tmp", bufs=2))
        scr = ctx.enter_context(tc.tile_pool(name="scr", bufs=4))
        accp = ctx.enter_context(tc.tile_pool(name="acc", bufs=1))

        acc = accp.tile([P, N_COLS * n_tiles], F32)

        for t in range(n_tiles):
            base = N_COLS * t

            s_t = inp.tile([P, f_tile], F32, tag="s")
            nc.gpsimd.dma_start(s_t[:], s_r[t])
            d_t = inp.tile([P, f_tile], F32, tag="d")
            nc.gpsimd.dma_start(d_t[:], d_r[t])
            dbp_t = inp.tile([P, f_tile], F32, tag="dbp")
            nc.gpsimd.dma_start(dbp_t[:], dbp_r[t])
            sbp_t = inp.tile([P, f_tile], F32, tag="sbp")
            nc.gpsimd.dma_start(sbp_t[:], sbp_r[t])

            # ---- smape halves: q = |pred - tgt| * 1/(pred + tgt) ----
            t1 = tmp.tile([P, f_tile], F32, tag="t1")
            nc.gpsimd.tensor_sub(t1[:], dbp_t[:], d_t[:])
            den1 = tmp.tile([P, f_tile], F32, tag="den1")
            nc.vector.tensor_add(den1[:], dbp_t[:], d_t[:])
            t2 = tmp.tile([P, f_tile], F32, tag="t2")
            nc.vector.tensor_sub(t2[:], sbp_t[:], s_t[:])
            den2 = tmp.tile([P, f_tile], F32, tag="den2")
            nc.gpsimd.tensor_add(den2[:], sbp_t[:], s_t[:])

            r1 = tmp.tile([P, f_tile], F32, tag="r1")
            nc.vector.reciprocal_approx_fast(out=r1[:], in_=den1[:])
            r2 = tmp.tile([P, f_tile], F32, tag="r2")
            nc.vector.reciprocal_approx_fast(out=r2[:], in_=den2[:])

            q1 = tmp.tile([P, f_tile], F32, tag="q1")
            nc.vector.scalar_tensor_tensor(
                q1[:], t1[:], 0.0, r1[:], op0=ALU.abs_max, op1=ALU.mult,
                accum_out=acc[:, base + 0 : base + 1],
            )
            q2 = tmp.tile([P, f_tile], F32, tag="q2")
            nc.vector.scalar_tensor_tensor(
                q2[:], t2[:], 0.0, r2[:], op0=ALU.abs_max, op1=ALU.mult,
                accum_out=acc[:, base + 1 : base + 2],
            )
            pe = tmp.tile([P, f_tile], F32, tag="pe")
            nc.vector.tensor_add(pe[:], q1[:], q2[:])

            # ---- mask margin tiles ----
            sd80 = tmp.tile([P, f_tile], F32, tag="sd80")
            nc.vector.tensor_scalar(sd80[:], d_t[:], 80.0, None, op0=ALU.subtract)
            d120 = tmp.tile([P, f_tile], F32, tag="d120")
            nc.gpsimd.tensor_scalar(d120[:], d_t[:], 120.0, None, op0=ALU.subtract)
            u1 = tmp.tile([P, f_tile], F32, tag="u1")
            nc.vector.tensor_scalar(u1[:], s_t[:], 135.0, 0.0, op0=ALU.subtract, op1=ALU.abs_max)
            u2 = tmp.tile([P, f_tile], F32, tag="u2")
            nc.gpsimd.tensor_scalar(u2[:], d_t[:], 85.0, 0.0, op0=ALU.subtract, op1=ALU.abs_max)

            m1 = tmp.tile([P, f_tile], F32, tag="m1")
            nc.gpsimd.scalar_tensor_tensor(
                m1[:], s_t[:], 120.0, sd80[:], op0=ALU.subtract, op1=ALU.max
            )
            m2 = tmp.tile([P, f_tile], F32, tag="m2")
            nc.vector.scalar_tensor_tensor(
                m2[:], s_t[:], 130.0, sd80[:], op0=ALU.subtract, op1=ALU.max
            )
            m3 = tmp.tile([P, f_tile], F32, tag="m3")
            nc.gpsimd.tensor_tensor(m3[:], u1[:], u2[:], op=ALU.min)
            m4 = tmp.tile([P, f_tile], F32, tag="m4")
            nc.vector.scalar_tensor_tensor(
                m4[:], s_t[:], 180.0, d120[:], op0=ALU.subtract, op1=ALU.max
            )

            # ---- masked sums (indicator * pe, fused accumulate) ----
            for ci, (m, thr, op) in enumerate(
                [
                    (m1, 0.0, ALU.is_lt),
                    (m2, 0.0, ALU.is_lt),
                    (m3, 5.0, ALU.is_lt),
                    (m4, 0.0, ALU.is_gt),
                ]
            ):
                o = scr.tile([P, f_tile], F32, tag="scr")
                eng = nc.vector if ci % 2 == 0 else nc.gpsimd
                eng.scalar_tensor_tensor(
                    o[:], m[:], thr, pe[:], op0=op, op1=ALU.mult,
                    accum_out=acc[:, base + 2 + ci : base + 3 + ci],
                )

            # ---- mask counts (indicator, fused accumulate) ----
            for ci, (m, thr, op) in enumerate(
                [
                    (m1, 0.0, ALU.is_lt),
                    (m2, 0.0, ALU.is_lt),
                    (m3, 5.0, ALU.is_lt),
                    (m4, 0.0, ALU.is_gt),
                ]
            ):
                o = scr.tile([P, f_tile], F32, tag="scr")
                eng = nc.gpsimd if ci % 2 == 0 else nc.vector
                eng.scalar_tensor_tensor(
                    o[:], m[:], thr, m[:], op0=op, op1=ALU.bypass,
                    accum_out=acc[:, base + 6 + ci : base + 7 + ci],
                )

        nc.gpsimd.dma_start(out_e[:], acc[:])

    return _split_excess_waits(nc) if split else nc


def _build_nc_v2(per_n=PER, f_tile=1024, split=True):
    """v2r: compare-path indicators (bf16) + TensorE diagonal-matmul for the
    masked sums and counts. pe is stored in 130-column blocks (128 data cols +
    a ones column + pad) so one matmul per (mask, block) yields both the
    masked-sum diagonal and the count column. Reciprocal + |.| with fused
    sum accumulation run on the Scalar engine."""
    n_tiles = per_n // (P * f_tile)
    n_blk = f_tile // P
    blkw = 130  # 128 data + 1 ones + 1 pad (4B alignment for bf16 2x mode)
    assert n_tiles * P * f_tile == per_n

    BF16 = mybir.dt.bfloat16
    nc = bass.Bass()
    s_e = nc.declare_dram_parameter("s", [per_n], F32, isOutput=False)
    d_e = nc.declare_dram_parameter("d", [per_n], F32, isOutput=False)
    dbp_e = nc.declare_dram_parameter("dbp", [per_n], F32, isOutput=False)
    sbp_e = nc.declare_dram_parameter("sbp", [per_n], F32, isOutput=False)
    out_q = nc.declare_dram_parameter("out", [P, 2 * n_tiles], F32, isOutput=True)
    out_d = nc.declare_dram_parameter("outd", [4, P, 129], F32, isOutput=True)

    s_r = s_e.rearrange("(t p f) -> t p f", p=P, f=f_tile)
    d_r = d_e.rearrange("(t p f) -> t p f", p=P, f=f_tile)
    dbp_r = dbp_e.rearrange("(t p f) -> t p f", p=P, f=f_tile)
    sbp_r = sbp_e.rearrange("(t p f) -> t p f", p=P, f=f_tile)

    with ExitStack() as ctx:
        tc = ctx.enter_context(tile.TileContext(nc))
        inp = ctx.enter_context(tc.tile_pool(name="inp", bufs=3))
        tmp = ctx.enter_context(tc.tile_pool(name="tmp", bufs=2))
        accp = ctx.enter_context(tc.tile_pool(name="acc", bufs=1))
        psum = ctx.enter_context(tc.tile_pool(name="psum", bufs=1, space="PSUM"))

        diag = [
            psum.tile([P, blkw], F32, tag=f"diag{i}", name=f"diag{i}")
            for i in range(4)
        ]
        acc = accp.tile([P, 2 * n_tiles], F32)
        bias_t = accp.tile([P, 3], F32)
        nc.gpsimd.memset(bias_t[:, 0:1], -135.0)
        nc.gpsimd.memset(bias_t[:, 1:2], -85.0)
        nc.gpsimd.memset(bias_t[:, 2:3], -120.0)

        def act_recip(out_ap, in_ap):
            nc.scalar.add_instruction(
                mybir.InstActivation(
                    name=nc.get_next_instruction_name(),
                    func=ACTF.Reciprocal,
                    ins=[
                        nc.scalar.lower_ap(in_ap),
                        mybir.ImmediateValue(dtype=F32, value=0.0),
                        mybir.ImmediateValue(dtype=F32, value=1.0),
                        mybir.ImmediateValue(dtype=F32, value=0.0),
                    ],
                    outs=[nc.scalar.lower_ap(out_ap)],
                )
            )

        for t in range(n_tiles):
            s_t = inp.tile([P, f_tile], F32, tag="s")
            nc.gpsimd.dma_start(s_t[:], s_r[t])
            d_t = inp.tile([P, f_tile], F32, tag="d")
            nc.gpsimd.dma_start(d_t[:], d_r[t])
            dbp_t = inp.tile([P, f_tile], F32, tag="dbp")
            nc.gpsimd.dma_start(dbp_t[:], dbp_r[t])
            sbp_t = inp.tile([P, f_tile], F32, tag="sbp")
            nc.gpsimd.dma_start(sbp_t[:], sbp_r[t])

            # ---- smape halves: q = |(pred - tgt) * recip(pred + tgt)| ----
            t1 = tmp.tile([P, f_tile], F32, tag="t1")
            nc.gpsimd.tensor_sub(t1[:], dbp_t[:], d_t[:])
            den1 = tmp.tile([P, f_tile], F32, tag="den1")
            nc.vector.tensor_add(den1[:], dbp_t[:], d_t[:])
            t2 = tmp.tile([P, f_tile], F32, tag="t2")
            nc.vector.tensor_sub(t2[:], sbp_t[:], s_t[:])
            den2 = tmp.tile([P, f_tile], F32, tag="den2")
            nc.gpsimd.tensor_add(den2[:], sbp_t[:], s_t[:])

            r1 = tmp.tile([P, f_tile], F32, tag="r1")
            act_recip(r1[:], den1[:])
            r2 = tmp.tile([P, f_tile], F32, tag="r2")
            act_recip(r2[:], den2[:])

            w1 = tmp.tile([P, f_tile], F32, tag="w1")
            nc.vector.tensor_mul(w1[:], t1[:], r1[:])
            w2 = tmp.tile([P, f_tile], F32, tag="w2")
            nc.vector.tensor_mul(w2[:], t2[:], r2[:])
            # q1 in 130-col blocks: cols k*130..k*130+127 data, k*130+128 ones
            aq1 = tmp.tile([P, n_blk * blkw], BF16, tag="aq1")
            aq1_3 = aq1[:].rearrange("p (b w) -> p b w", w=blkw)
            nc.vector.memset(aq1_3[:, :, 128:130], 1.0)
            nc.scalar.activation(
                aq1_3[:, :, 0:128], w1[:].rearrange("p (b w) -> p b w", w=P),
                ACTF.Abs, accum_out=acc[:, 2 * t : 2 * t + 1]
            )
            aq2 = tmp.tile([P, f_tile], BF16, tag="aq2")
            nc.scalar.activation(
                aq2[:], w2[:], ACTF.Abs, accum_out=acc[:, 2 * t + 1 : 2 * t + 2]
            )

            # ---- indicators (bf16, sign-exact) ----
            c1 = tmp.tile([P, f_tile], BF16, tag="c1")
            nc.vector.tensor_scalar(c1[:], s_t[:], 120.0, None, op0=ALU.is_lt)
            c2 = tmp.tile([P, f_tile], BF16, tag="c2")
            nc.vector.tensor_scalar(c2[:], s_t[:], 130.0, None, op0=ALU.is_lt)
            g1 = tmp.tile([P, f_tile], BF16, tag="g1")
            nc.vector.tensor_scalar(g1[:], d_t[:], 80.0, None, op0=ALU.is_lt)
            u1 = tmp.tile([P, f_tile], F32, tag="u1")
            nc.scalar.activation(u1[:], s_t[:], ACTF.Abs, bias=bias_t[:, 0:1])
            u2 = tmp.tile([P, f_tile], F32, tag="u2")
            nc.scalar.activation(u2[:], d_t[:], ACTF.Abs, bias=bias_t[:, 1:2])
            c4 = tmp.tile([P, f_tile], BF16, tag="c4")
            nc.vector.tensor_scalar(c4[:], s_t[:], 180.0, None, op0=ALU.is_gt)
            g3 = tmp.tile([P, f_tile], BF16, tag="g3")
            nc.scalar.activation(g3[:], d_t[:], ACTF.Sign, bias=bias_t[:, 2:3])

            p1m = tmp.tile([P, f_tile], BF16, tag="p1m")
            nc.gpsimd.tensor_mul(p1m[:], c1[:], g1[:])
            p2m = tmp.tile([P, f_tile], BF16, tag="p2m")
            nc.vector.tensor_mul(p2m[:], c2[:], g1[:])
            m3 = tmp.tile([P, f_tile], F32, tag="m3")
            nc.vector.tensor_tensor(m3[:], u1[:], u2[:], op=ALU.min)
            h1m = tmp.tile([P, f_tile], BF16, tag="h1m")
            nc.vector.tensor_scalar(h1m[:], m3[:], 5.0, None, op0=ALU.is_lt)
            crm = tmp.tile([P, f_tile], BF16, tag="crm")
            nc.vector.tensor_tensor(crm[:], c4[:], g3[:], op=ALU.max)

            for mi, mk in enumerate([p1m, p2m, h1m, crm]):
                for blk in range(n_blk):
                    first = t == 0 and blk == 0
                    last = t == n_tiles - 1 and blk == n_blk - 1
                    lo = blk * P
                    nc.tensor.matmul(
                        diag[mi][:, 0:129],
                        mk[:, lo : lo + P],
                        aq1[:, blk * blkw : blk * blkw + 129],
                        start=first,
                        stop=False,
                    )
                    nc.tensor.matmul(
                        diag[mi][:, 0:128],
                        mk[:, lo : lo + P],
                        aq2[:, lo : lo + P],
                        start=False,
                        stop=last,
                    )

        stage_d = accp.tile([P, 4 * 129], F32)
        for i in range(4):
            nc.vector.tensor_copy(stage_d[:, i * 129 : (i + 1) * 129], diag[i][:, 0:129])
            nc.gpsimd.dma_start(out_d[i], stage_d[:, i * 129 : (i + 1) * 129])
        nc.gpsimd.dma_start(out_q[:], acc[:])

    return _split_excess_waits(nc) if split else nc


VERSION = 2


def _get_nc(per_n=PER, f_tile=None):
    if f_tile is None:
        f_tile = 1024 if VERSION == 2 else F_TILE
    key = (VERSION, per_n, f_tile)
    if key not in _NC_CACHE:
        builder = _build_nc_v2 if VERSION == 2 else _build_nc
        _NC_CACHE[key] = builder(per_n, f_tile)
    return _NC_CACHE[key]


def _finalize(vec10, batch_n):
    """Host-side: combine the 10 global partial sums into the loss (f64)."""
    sq1, sq2, s_p1, s_p2, s_h1, s_cr, c_p1, c_p2, c_h1, c_cr = [
        float(x) for x in vec10
    ]
    s_tot = sq1 + sq2
    # factor 2 from smape definition
    S = [
        2.0 * s_p1,                       # normal
        2.0 * (s_p2 - s_p1),              # elevated
        2.0 * s_h1,                       # hyper1
        2.0 * (s_tot - s_p2 - s_h1),      # hyper2
        2.0 * s_cr,                       # crisis
    ]
    C = [
        c_p1,
        c_p2 - c_p1,
        c_h1,
        batch_n - c_p2 - c_h1,
        c_cr,
    ]
    rst = 0.0
    m_rst = 0.0
    mask_cnt = 0
    for s_m, cnt in zip(S, C):
        w = np.sqrt(np.log(batch_n / max(cnt, 1.0)))
        if cnt > 0:
            m_rst = (m_rst + s_m * w) / cnt / 2.0
            rst = rst + m_rst
            mask_cnt += 1
    if mask_cnt == 0:
        return rst / 5.0
    return rst / mask_cnt


def host_partials(s, d, dbp, sbp):
    """Numpy replica of the device partials (for testing)."""
    s = s.astype(np.float64)
    d = d.astype(np.float64)
    dbp = dbp.astype(np.float64)
    sbp = sbp.astype(np.float64)
    q1 = np.abs(dbp - d) / (dbp + d)
    q2 = np.abs(sbp - s) / (sbp + s)
    pe = q1 + q2
    m1 = np.maximum(s - 120, d - 80)
    m2 = np.maximum(s - 130, d - 80)
    m3 = np.minimum(np.abs(s - 135), np.abs(d - 85))
    m4 = np.maximum(s - 180, d - 120)
    return np.array(
        [
            q1.sum(),
            q2.sum(),
            pe[m1 < 0].sum(),
            pe[m2 < 0].sum(),
            pe[m3 < 5].sum(),
            pe[m4 > 0].sum(),
            (m1 < 0).sum(),
            (m2 < 0).sum(),
            (m3 < 5).sum(),
            (m4 > 0).sum(),
        ]
    )


def kernel(**inputs):
    s = np.ascontiguousarray(np.asarray(inputs["s"], dtype=np.float32).reshape(-1))
    d = np.ascontiguousarray(np.asarray(inputs["d"], dtype=np.float32).reshape(-1))
    dbp = np.ascontiguousarray(
        np.asarray(inputs["dbp_pred"], dtype=np.float32).reshape(-1)
    )
    sbp = np.ascontiguousarray(
        np.asarray(inputs["sbp_pred"], dtype=np.float32).reshape(-1)
    )
    batch_n = s.shape[0]
    assert batch_n == B, f"expected {B}, got {batch_n}"

    nc = _get_nc()
    in_maps = []
    for c in range(NCORES):
        sl = slice(c * PER, (c + 1) * PER)
        in_maps.append({"s": s[sl], "d": d[sl], "dbp": dbp[sl], "sbp": sbp[sl]})

    res = run_bass_kernel_spmd(nc, in_maps, list(range(NCORES)), trace=TRACE)
    LAST_RESULT["exec_time_ns"] = res.exec_time_ns
    LAST_RESULT["raw"] = res

    tot = np.zeros(N_COLS, np.float64)
    for r in res.results:
        if VERSION == 2:
            q = np.asarray(r["out"], np.float64)  # [P, 2*n_tiles]
            diag = np.asarray(r["outd"], np.float64)  # [4, P, 130]
            tot[0] += q[:, 0::2].sum()
            tot[1] += q[:, 1::2].sum()
            for i in range(4):
                tot[2 + i] += np.trace(diag[i, :, 0:128])
                tot[6 + i] += diag[i, :, 128].sum()
        else:
            o = np.asarray(r["out"], np.float64).reshape(P, N_TILES, N_COLS)
            tot += o.sum(axis=(0, 1))
    loss = _finalize(tot, float(batch_n))
    return np.float32(loss)


# revision 30
# speedup vs baseline: 1.1193x; 1.0306x over previous
"""Trainium2 Bass kernel for nn_AmpLoss_87754771792112.

Strategy: pure data-parallel across 8 NeuronCores. Each core processes a
contiguous 1/8 batch shard and emits per-partition partial sums:
  - Sq1 = sum |dbp-d| /(dbp+d)   (smape halves, factor 2 applied on host)
  - Sq2 = sum |sbp-s| /(sbp+s)
  - masked smape sums and counts for the masks:
      P1  = normal          = (s<120)&(d<80)        == max(s-120, d-80) < 0
      P2  = normal|elevated = (s<130)&(d<80)        == max(s-130, d-80) < 0
      h1  = hyper1          = (130<=s<140)|(80<=d<90) minus (normal|elev)
                            == min(|s-135|, |d-85|) < 5   (exclusion automatic)
      cr  = crisis          = (s>180)|(d>120)       == max(s-180, d-120) > 0
  hyper2 stats are recovered on the host by subtraction (the four
  non-crisis masks partition the space).

The tiny 5-mask sequential accumulation runs on the host in float64
during the gather step.

Only 4 of the 6 inputs are read (m / mbp_pred are dead in the loss).
"""

import numpy as np

try:
    import concourse.bass as bass
except ImportError:  # grading container path
    import sys

    sys.path.insert(0, "/opt/trn_rl_repo")
    import concourse.bass as bass

from contextlib import ExitStack

import concourse.tile as tile
from concourse import mybir
from concourse.bass_utils import run_bass_kernel_spmd

F32 = mybir.dt.float32
ALU = mybir.AluOpType
ACTF = mybir.ActivationFunctionType

B = 16777216
NCORES = 8
PER = B // NCORES  # 2097152
P = 128
F_TILE = 512
N_TILES = PER // (P * F_TILE)  # 16
N_COLS = 10  # accumulator columns per tile iteration

TRACE = False  # set True from test.py for neuron-profile timing
LAST_RESULT = {}

_NC_CACHE = {}


def _split_excess_waits(nc, max_waits=1):
    """This walrus build rejects >1 sync wait on one instruction. Spill the
    excess onto Drain instructions inserted just before, on the same engine."""
    for fn in nc.m.functions:
        for blk in fn.blocks:
            out = []
            for inst in blk.instructions:
                si = inst.sync_info
                if si is not None and si.on_wait and len(si.on_wait) > max_waits:
                    waits = list(si.on_wait)
                    keep, spill = waits[:max_waits], waits[max_waits:]
                    k = 0
                    while spill:
                        chunk, spill = spill[:max_waits], spill[max_waits:]
                        nop = mybir.InstDrain(
                            name=f"{inst.name}-w{k}", engine=inst.engine
                        )
                        nop.sync_info = mybir.SyncInfo(on_wait=chunk, on_update=[])
                        out.append(nop)
                        k += 1
                    inst.sync_info = mybir.SyncInfo(
                        on_wait=keep, on_update=list(si.on_update or [])
                    )
                out.append(inst)
            blk.instructions = out
    return nc


def _build_nc(per_n=PER, f_tile=F_TILE, split=True):
    """Build the single-core Bass graph (same graph runs SPMD on all cores)."""
    n_tiles = per_n // (P * f_tile)
    assert n_tiles * P * f_tile == per_n

    nc = bass.Bass()
    s_e = nc.declare_dram_parameter("s", [per_n], F32, isOutput=False)
    d_e = nc.declare_dram_parameter("d", [per_n], F32, isOutput=False)
    dbp_e = nc.declare_dram_parameter("dbp", [per_n], F32, isOutput=False)
    sbp_e = nc.declare_dram_parameter("sbp", [per_n], F32, isOutput=False)
    out_e = nc.declare_dram_parameter("out", [P, N_COLS * n_tiles], F32, isOutput=True)

    s_r = s_e.rearrange("(t p f) -> t p f", p=P, f=f_tile)
    d_r = d_e.rearrange("(t p f) -> t p f", p=P, f=f_tile)
    dbp_r = dbp_e.rearrange("(t p f) -> t p f", p=P, f=f_tile)
    sbp_r = sbp_e.rearrange("(t p f) -> t p f", p=P, f=f_tile)

    with ExitStack() as ctx:
        tc = ctx.enter_context(tile.TileContext(nc))
        inp = ctx.enter_context(tc.tile_pool(name="inp", bufs=2))
        tmp = ctx.enter_context(tc.tile_pool(name="tmp", bufs=2))
        scr = ctx.enter_context(tc.tile_pool(name="scr", bufs=4))
        accp = ctx.enter_context(tc.tile_pool(name="acc", bufs=1))

        acc = accp.tile([P, N_COLS * n_tiles], F32)

        for t in range(n_tiles):
            base = N_COLS * t

            s_t = inp.tile([P, f_tile], F32, tag="s")
            nc.gpsimd.dma_start(s_t[:], s_r[t])
            d_t = inp.tile([P, f_tile], F32, tag="d")
            nc.gpsimd.dma_start(d_t[:], d_r[t])
            dbp_t = inp.tile([P, f_tile], F32, tag="dbp")
            nc.gpsimd.dma_start(dbp_t[:], dbp_r[t])
            sbp_t = inp.tile([P, f_tile], F32, tag="sbp")
            nc.gpsimd.dma_start(sbp_t[:], sbp_r[t])

            # ---- smape halves: q = |pred - tgt| * 1/(pred + tgt) ----
            t1 = tmp.tile([P, f_tile], F32, tag="t1")
            nc.gpsimd.tensor_sub(t1[:], dbp_t[:], d_t[:])
            den1 = tmp.tile([P, f_tile], F32, tag="den1")
            nc.vector.tensor_add(den1[:], dbp_t[:], d_t[:])
            t2 = tmp.tile([P, f_tile], F32, tag="t2")
            nc.vector.tensor_sub(t2[:], sbp_t[:], s_t[:])
            den2 = tmp.tile([P, f_tile], F32, tag="den2")
            nc.gpsimd.tensor_add(den2[:], sbp_t[:], s_t[:])

            r1 = tmp.tile([P, f_tile], F32, tag="r1")
            nc.vector.reciprocal_approx_fast(out=r1[:], in_=den1[:])
            r2 = tmp.tile([P, f_tile], F32, tag="r2")
            nc.vector.reciprocal_approx_fast(out=r2[:], in_=den2[:])

            q1 = tmp.tile([P, f_tile], F32, tag="q1")
            nc.vector.scalar_tensor_tensor(
                q1[:], t1[:], 0.0, r1[:], op0=ALU.abs_max, op1=ALU.mult,
                accum_out=acc[:, base + 0 : base + 1],
            )
            q2 = tmp.tile([P, f_tile], F32, tag="q2")
            nc.vector.scalar_tensor_tensor(
                q2[:], t2[:], 0.0, r2[:], op0=ALU.abs_max, op1=ALU.mult,
                accum_out=acc[:, base + 1 : base + 2],
            )
            pe = tmp.tile([P, f_tile], F32, tag="pe")
            nc.vector.tensor_add(pe[:], q1[:], q2[:])

            # ---- mask margin tiles ----
            sd80 = tmp.tile([P, f_tile], F32, tag="sd80")
            nc.vector.tensor_scalar(sd80[:], d_t[:], 80.0, None, op0=ALU.subtract)
            d120 = tmp.tile([P, f_tile], F32, tag="d120")
            nc.gpsimd.tensor_scalar(d120[:], d_t[:], 120.0, None, op0=ALU.subtract)
            u1 = tmp.tile([P, f_tile], F32, tag="u1")
            nc.vector.tensor_scalar(u1[:], s_t[:], 135.0, 0.0, op0=ALU.subtract, op1=ALU.abs_max)
            u2 = tmp.tile([P, f_tile], F32, tag="u2")
            nc.gpsimd.tensor_scalar(u2[:], d_t[:], 85.0, 0.0, op0=ALU.subtract, op1=ALU.abs_max)

            m1 = tmp.tile([P, f_tile], F32, tag="m1")
            nc.gpsimd.scalar_tensor_tensor(
                m1[:], s_t[:], 120.0, sd80[:], op0=ALU.subtract, op1=ALU.max
            )
            m2 = tmp.tile([P, f_tile], F32, tag="m2")
            nc.vector.scalar_tensor_tensor(
                m2[:], s_t[:], 130.0, sd80[:], op0=ALU.subtract, op1=ALU.max
            )
            m3 = tmp.tile([P, f_tile], F32, tag="m3")
            nc.gpsimd.tensor_tensor(m3[:], u1[:], u2[:], op=ALU.min)
            m4 = tmp.tile([P, f_tile], F32, tag="m4")
            nc.vector.scalar_tensor_tensor(
                m4[:], s_t[:], 180.0, d120[:], op0=ALU.subtract, op1=ALU.max
            )

            # ---- masked sums (indicator * pe, fused accumulate) ----
            for ci, (m, thr, op) in enumerate(
                [
                    (m1, 0.0, ALU.is_lt),
                    (m2, 0.0, ALU.is_lt),
                    (m3, 5.0, ALU.is_lt),
                    (m4, 0.0, ALU.is_gt),
                ]
            ):
                o = scr.tile([P, f_tile], F32, tag="scr")
                eng = nc.vector if ci % 2 == 0 else nc.gpsimd
                eng.scalar_tensor_tensor(
                    o[:], m[:], thr, pe[:], op0=op, op1=ALU.mult,
                    accum_out=acc[:, base + 2 + ci : base + 3 + ci],
                )

            # ---- mask counts (indicator, fused accumulate) ----
            for ci, (m, thr, op) in enumerate(
                [
                    (m1, 0.0, ALU.is_lt),
                    (m2, 0.0, ALU.is_lt),
                    (m3, 5.0, ALU.is_lt),
                    (m4, 0.0, ALU.is_gt),
                ]
            ):
                o = scr.tile([P, f_tile], F32, tag="scr")
                eng = nc.gpsimd if ci % 2 == 0 else nc.vector
                eng.scalar_tensor_tensor(
                    o[:], m[:], thr, m[:], op0=op, op1=ALU.bypass,
                    accum_out=acc[:, base + 6 + ci : base + 7 + ci],
                )

        nc.gpsimd.dma_start(out_e[:], acc[:])

    return _split_excess_waits(nc) if split else nc


def _build_nc_v2(per_n=PER, f_tile=1024, split=True):
    """v2r: compare-path indicators (bf16) + TensorE diagonal-matmul for the
    masked sums and counts. pe is stored in 130-column blocks (128 data cols +
    a ones column + pad) so one matmul per (mask, block) yields both the
    masked-sum diagonal and the count column. Reciprocal + |.| with fused
    sum accumulation run on the Scalar engine."""
    n_tiles = per_n // (P * f_tile)
    n_blk = f_tile // P
    blkw = 130  # 128 data + 1 ones + 1 pad (4B alignment for bf16 2x mode)
    assert n_tiles * P * f_tile == per_n

    BF16 = mybir.dt.bfloat16
    nc = bass.Bass()
    s_e = nc.declare_dram_parameter("s", [per_n], F32, isOutput=False)
    d_e = nc.declare_dram_parameter("d", [per_n], F32, isOutput=False)
    dbp_e = nc.declare_dram_parameter("dbp", [per_n], F32, isOutput=False)
    sbp_e = nc.declare_dram_parameter("sbp", [per_n], F32, isOutput=False)
    out_q = nc.declare_dram_parameter("out", [P, 2 * n_tiles], F32, isOutput=True)
    out_d = nc.declare_dram_parameter("outd", [4, P, 129], F32, isOutput=True)

    s_r = s_e.rearrange("(t p f) -> t p f", p=P, f=f_tile)
    d_r = d_e.rearrange("(t p f) -> t p f", p=P, f=f_tile)
    dbp_r = dbp_e.rearrange("(t p f) -> t p f", p=P, f=f_tile)
    sbp_r = sbp_e.rearrange("(t p f) -> t p f", p=P, f=f_tile)

    with ExitStack() as ctx:
        tc = ctx.enter_context(tile.TileContext(nc))
        inp = ctx.enter_context(tc.tile_pool(name="inp", bufs=3))
        tmp = ctx.enter_context(tc.tile_pool(name="tmp", bufs=2))
        accp = ctx.enter_context(tc.tile_pool(name="acc", bufs=1))
        psum = ctx.enter_context(tc.tile_pool(name="psum", bufs=1, space="PSUM"))

        diag = [
            psum.tile([P, blkw], F32, tag=f"diag{i}", name=f"diag{i}")
            for i in range(4)
        ]
        acc = accp.tile([P, 2 * n_tiles], F32)
        bias_t = accp.tile([P, 3], F32)
        nc.gpsimd.memset(bias_t[:, 0:1], -135.0)
        nc.gpsimd.memset(bias_t[:, 1:2], -85.0)
        nc.gpsimd.memset(bias_t[:, 2:3], -120.0)

        def act_recip(out_ap, in_ap):
            nc.scalar.add_instruction(
                mybir.InstActivation(
                    name=nc.get_next_instruction_name(),
                    func=ACTF.Reciprocal,
                    ins=[
                        nc.scalar.lower_ap(in_ap),
                        mybir.ImmediateValue(dtype=F32, value=0.0),
                        mybir.ImmediateValue(dtype=F32, value=1.0),
                        mybir.ImmediateValue(dtype=F32, value=0.0),
                    ],
                    outs=[nc.scalar.lower_ap(out_ap)],
                )
            )

        for t in range(n_tiles):
            s_t = inp.tile([P, f_tile], F32, tag="s")
            nc.sync.dma_start(s_t[:], s_r[t])
            d_t = inp.tile([P, f_tile], F32, tag="d")
            nc.sync.dma_start(d_t[:], d_r[t])
            dbp_t = inp.tile([P, f_tile], F32, tag="dbp")
            nc.sync.dma_start(dbp_t[:], dbp_r[t])
            sbp_t = inp.tile([P, f_tile], F32, tag="sbp")
            nc.sync.dma_start(sbp_t[:], sbp_r[t])

            # ---- smape halves: q = |(pred - tgt) * recip(pred + tgt)| ----
            t1 = tmp.tile([P, f_tile], F32, tag="t1")
            nc.gpsimd.tensor_sub(t1[:], dbp_t[:], d_t[:])
            den1 = tmp.tile([P, f_tile], F32, tag="den1")
            nc.vector.tensor_add(den1[:], dbp_t[:], d_t[:])
            t2 = tmp.tile([P, f_tile], F32, tag="t2")
            nc.vector.tensor_sub(t2[:], sbp_t[:], s_t[:])
            den2 = tmp.tile([P, f_tile], F32, tag="den2")
            nc.gpsimd.tensor_add(den2[:], sbp_t[:], s_t[:])

            r1 = tmp.tile([P, f_tile], F32, tag="r1")
            act_recip(r1[:], den1[:])
            r2 = tmp.tile([P, f_tile], F32, tag="r2")
            act_recip(r2[:], den2[:])

            w1 = tmp.tile([P, f_tile], F32, tag="w1")
            nc.vector.tensor_mul(w1[:], t1[:], r1[:])
            w2 = tmp.tile([P, f_tile], F32, tag="w2")
            nc.vector.tensor_mul(w2[:], t2[:], r2[:])
            # q1 in 130-col blocks: cols k*130..k*130+127 data, k*130+128 ones
            aq1 = tmp.tile([P, n_blk * blkw], BF16, tag="aq1")
            aq1_3 = aq1[:].rearrange("p (b w) -> p b w", w=blkw)
            nc.vector.memset(aq1_3[:, :, 128:130], 1.0)
            nc.scalar.activation(
                aq1_3[:, :, 0:128], w1[:].rearrange("p (b w) -> p b w", w=P),
                ACTF.Abs, accum_out=acc[:, 2 * t : 2 * t + 1]
            )
            aq2 = tmp.tile([P, f_tile], BF16, tag="aq2")
            nc.scalar.activation(
                aq2[:], w2[:], ACTF.Abs, accum_out=acc[:, 2 * t + 1 : 2 * t + 2]
            )

            # ---- indicators (bf16, sign-exact) ----
            c1 = tmp.tile([P, f_tile], BF16, tag="c1")
            nc.vector.tensor_scalar(c1[:], s_t[:], 120.0, None, op0=ALU.is_lt)
            c2 = tmp.tile([P, f_tile], BF16, tag="c2")
            nc.vector.tensor_scalar(c2[:], s_t[:], 130.0, None, op0=ALU.is_lt)
            g1 = tmp.tile([P, f_tile], BF16, tag="g1")
            nc.vector.tensor_scalar(g1[:], d_t[:], 80.0, None, op0=ALU.is_lt)
            u1 = tmp.tile([P, f_tile], F32, tag="u1")
            nc.scalar.activation(u1[:], s_t[:], ACTF.Abs, bias=bias_t[:, 0:1])
            u2 = tmp.tile([P, f_tile], F32, tag="u2")
            nc.scalar.activation(u2[:], d_t[:], ACTF.Abs, bias=bias_t[:, 1:2])
            c4 = tmp.tile([P, f_tile], BF16, tag="c4")
            nc.vector.tensor_scalar(c4[:], s_t[:], 180.0, None, op0=ALU.is_gt)
            g3 = tmp.tile([P, f_tile], BF16, tag="g3")
            nc.scalar.activation(g3[:], d_t[:], ACTF.Sign, bias=bias_t[:, 2:3])

            p1m = tmp.tile([P, f_tile], BF16, tag="p1m")
            nc.gpsimd.tensor_mul(p1m[:], c1[:], g1[:])
            p2m = tmp.tile([P, f_tile], BF16, tag="p2m")
            nc.vector.tensor_mul(p2m[:], c2[:], g1[:])
            m3 = tmp.tile([P, f_tile], F32, tag="m3")
            nc.vector.tensor_tensor(m3[:], u1[:], u2[:], op=ALU.min)
            h1m = tmp.tile([P, f_tile], BF16, tag="h1m")
            nc.vector.tensor_scalar(h1m[:], m3[:], 5.0, None, op0=ALU.is_lt)
            crm = tmp.tile([P, f_tile], BF16, tag="crm")
            nc.vector.tensor_tensor(crm[:], c4[:], g3[:], op=ALU.max)

            for mi, mk in enumerate([p1m, p2m, h1m, crm]):
                for blk in range(n_blk):
                    first = t == 0 and blk == 0
                    last = t == n_tiles - 1 and blk == n_blk - 1
                    lo = blk * P
                    nc.tensor.matmul(
                        diag[mi][:, 0:129],
                        mk[:, lo : lo + P],
                        aq1[:, blk * blkw : blk * blkw + 129],
                        start=first,
                        stop=False,
                    )
                    nc.tensor.matmul(
                        diag[mi][:, 0:128],
                        mk[:, lo : lo + P],
                        aq2[:, lo : lo + P],
                        start=False,
                        stop=last,
                    )

        stage_d = accp.tile([P, 4 * 129], F32)
        for i in range(4):
            nc.vector.tensor_copy(stage_d[:, i * 129 : (i + 1) * 129], diag[i][:, 0:129])
            nc.sync.dma_start(out_d[i], stage_d[:, i * 129 : (i + 1) * 129])
        nc.sync.dma_start(out_q[:], acc[:])

    return _split_excess_waits(nc) if split else nc


VERSION = 2


def _get_nc(per_n=PER, f_tile=None):
    if f_tile is None:
        f_tile = 1024 if VERSION == 2 else F_TILE
    key = (VERSION, per_n, f_tile)
    if key not in _NC_CACHE:
        builder = _build_nc_v2 if VERSION == 2 else _build_nc
        _NC_CACHE[key] = builder(per_n, f_tile)
    return _NC_CACHE[key]


def _finalize(vec10, batch_n):
    """Host-side: combine the 10 global partial sums into the loss (f64)."""
    sq1, sq2, s_p1, s_p2, s_h1, s_cr, c_p1, c_p2, c_h1, c_cr = [
        float(x) for x in vec10
    ]
    s_tot = sq1 + sq2
    # factor 2 from smape definition
    S = [
        2.0 * s_p1,                       # normal
        2.0 * (s_p2 - s_p1),              # elevated
        2.0 * s_h1,                       # hyper1
        2.0 * (s_tot - s_p2 - s_h1),      # hyper2
        2.0 * s_cr,                       # crisis
    ]
    C = [
        c_p1,
        c_p2 - c_p1,
        c_h1,
        batch_n - c_p2 - c_h1,
        c_cr,
    ]
    rst = 0.0
    m_rst = 0.0
    mask_cnt = 0
    for s_m, cnt in zip(S, C):
        w = np.sqrt(np.log(batch_n / max(cnt, 1.0)))
        if cnt > 0:
            m_rst = (m_rst + s_m * w) / cnt / 2.0
            rst = rst + m_rst
            mask_cnt += 1
    if mask_cnt == 0:
        return rst / 5.0
    return rst / mask_cnt


def host_partials(s, d, dbp, sbp):
    """Numpy replica of the device partials (for testing)."""
    s = s.astype(np.float64)
    d = d.astype(np.float64)
    dbp = dbp.astype(np.float64)
    sbp = sbp.astype(np.float64)
    q1 = np.abs(dbp - d) / (dbp + d)
    q2 = np.abs(sbp - s) / (sbp + s)
    pe = q1 + q2
    m1 = np.maximum(s - 120, d - 80)
    m2 = np.maximum(s - 130, d - 80)
    m3 = np.minimum(np.abs(s - 135), np.abs(d - 85))
    m4 = np.maximum(s - 180, d - 120)
    return np.array(
        [
            q1.sum(),
            q2.sum(),
            pe[m1 < 0].sum(),
            pe[m2 < 0].sum(),
            pe[m3 < 5].sum(),
            pe[m4 > 0].sum(),
            (m1 < 0).sum(),
            (m2 < 0).sum(),
            (m3 < 5).sum(),
            (m4 > 0).sum(),
        ]
    )


def kernel(**inputs):
    s = np.ascontiguousarray(np.asarray(inputs["s"], dtype=np.float32).reshape(-1))
    d = np.ascontiguousarray(np.asarray(inputs["d"], dtype=np.float32).reshape(-1))
    dbp = np.ascontiguousarray(
        np.asarray(inputs["dbp_pred"], dtype=np.float32).reshape(-1)
    )
    sbp = np.ascontiguousarray(
        np.asarray(inputs["sbp_pred"], dtype=np.float32).reshape(-1)
    )
    batch_n = s.shape[0]
    assert batch_n == B, f"expected {B}, got {batch_n}"

    nc = _get_nc()
    in_maps = []
    for c in range(NCORES):
        sl = slice(c * PER, (c + 1) * PER)
        in_maps.append({"s": s[sl], "d": d[sl], "dbp": dbp[sl], "sbp": sbp[sl]})

    res = run_bass_kernel_spmd(nc, in_maps, list(range(NCORES)), trace=TRACE)
    LAST_RESULT["exec_time_ns"] = res.exec_time_ns
    LAST_RESULT["raw"] = res

    tot = np.zeros(N_COLS, np.float64)
    for r in res.results:
        if VERSION == 2:
            q = np.asarray(r["out"], np.float64)  # [P, 2*n_tiles]
            diag = np.asarray(r["outd"], np.float64)  # [4, P, 130]
            tot[0] += q[:, 0::2].sum()
            tot[1] += q[:, 1::2].sum()
            for i in range(4):
                tot[2 + i] += np.trace(diag[i, :, 0:128])
                tot[6 + i] += diag[i, :, 128].sum()
        else:
            o = np.asarray(r["out"], np.float64).reshape(P, N_TILES, N_COLS)
            tot += o.sum(axis=(0, 1))
    loss = _finalize(tot, float(batch_n))
    return np.float32(loss)


# revision 31
# speedup vs baseline: 1.2106x; 1.0816x over previous
"""Trainium2 Bass kernel for nn_AmpLoss_87754771792112.

Strategy: pure data-parallel across 8 NeuronCores. Each core processes a
contiguous 1/8 batch shard and emits per-partition partial sums:
  - Sq1 = sum |dbp-d| /(dbp+d)   (smape halves, factor 2 applied on host)
  - Sq2 = sum |sbp-s| /(sbp+s)
  - masked smape sums and counts for the masks:
      P1  = normal          = (s<120)&(d<80)        == max(s-120, d-80) < 0
      P2  = normal|elevated = (s<130)&(d<80)        == max(s-130, d-80) < 0
      h1  = hyper1          = (130<=s<140)|(80<=d<90) minus (normal|elev)
                            == min(|s-135|, |d-85|) < 5   (exclusion automatic)
      cr  = crisis          = (s>180)|(d>120)       == max(s-180, d-120) > 0
  hyper2 stats are recovered on the host by subtraction (the four
  non-crisis masks partition the space).

The tiny 5-mask sequential accumulation runs on the host in float64
during the gather step.

Only 4 of the 6 inputs are read (m / mbp_pred are dead in the loss).
"""

import numpy as np

try:
    import concourse.bass as bass
except ImportError:  # grading container path
    import sys

    sys.path.insert(0, "/opt/trn_rl_repo")
    import concourse.bass as bass

from contextlib import ExitStack

import concourse.tile as tile
from concourse import mybir
from concourse.bass_utils import run_bass_kernel_spmd

F32 = mybir.dt.float32
ALU = mybir.AluOpType
ACTF = mybir.ActivationFunctionType

B = 16777216
NCORES = 8
PER = B // NCORES  # 2097152
P = 128
F_TILE = 512
N_TILES = PER // (P * F_TILE)  # 16
N_COLS = 10  # accumulator columns per tile iteration

TRACE = False  # set True from test.py for neuron-profile timing
LAST_RESULT = {}

_NC_CACHE = {}


def _split_excess_waits(nc, max_waits=1):
    """This walrus build rejects >1 sync wait on one instruction. Spill the
    excess onto Drain instructions inserted just before, on the same engine."""
    for fn in nc.m.functions:
        for blk in fn.blocks:
            out = []
            for inst in blk.instructions:
                si = inst.sync_info
                if si is not None and si.on_wait and len(si.on_wait) > max_waits:
                    waits = list(si.on_wait)
                    keep, spill = waits[:max_waits], waits[max_waits:]
                    k = 0
                    while spill:
                        chunk, spill = spill[:max_waits], spill[max_waits:]
                        nop = mybir.InstDrain(
                            name=f"{inst.name}-w{k}", engine=inst.engine
                        )
                        nop.sync_info = mybir.SyncInfo(on_wait=chunk, on_update=[])
                        out.append(nop)
                        k += 1
                    inst.sync_info = mybir.SyncInfo(
                        on_wait=keep, on_update=list(si.on_update or [])
                    )
                out.append(inst)
            blk.instructions = out
    return nc


def _build_nc(per_n=PER, f_tile=F_TILE, split=True):
    """Build the single-core Bass graph (same graph runs SPMD on all cores)."""
    n_tiles = per_n // (P * f_tile)
    assert n_tiles * P * f_tile == per_n

    nc = bass.Bass()
    s_e = nc.declare_dram_parameter("s", [per_n], F32, isOutput=False)
    d_e = nc.declare_dram_parameter("d", [per_n], F32, isOutput=False)
    dbp_e = nc.declare_dram_parameter("dbp", [per_n], F32, isOutput=False)
    sbp_e = nc.declare_dram_parameter("sbp", [per_n], F32, isOutput=False)
    out_e = nc.declare_dram_parameter("out", [P, N_COLS * n_tiles], F32, isOutput=True)

    s_r = s_e.rearrange("(t p f) -> t p f", p=P, f=f_tile)
    d_r = d_e.rearrange("(t p f) -> t p f", p=P, f=f_tile)
    dbp_r = dbp_e.rearrange("(t p f) -> t p f", p=P, f=f_tile)
    sbp_r = sbp_e.rearrange("(t p f) -> t p f", p=P, f=f_tile)

    with ExitStack() as ctx:
        tc = ctx.enter_context(tile.TileContext(nc))
        inp = ctx.enter_context(tc.tile_pool(name="inp", bufs=2))
        tmp = ctx.enter_context(tc.tile_pool(name="tmp", bufs=2))
        scr = ctx.enter_context(tc.tile_pool(name="scr", bufs=4))
        accp = ctx.enter_context(tc.tile_pool(name="acc", bufs=1))

        acc = accp.tile([P, N_COLS * n_tiles], F32)

        for t in range(n_tiles):
            base = N_COLS * t

            s_t = inp.tile([P, f_tile], F32, tag="s")
            nc.gpsimd.dma_start(s_t[:], s_r[t])
            d_t = inp.tile([P, f_tile], F32, tag="d")
            nc.gpsimd.dma_start(d_t[:], d_r[t])
            dbp_t = inp.tile([P, f_tile], F32, tag="dbp")
            nc.gpsimd.dma_start(dbp_t[:], dbp_r[t])
            sbp_t = inp.tile([P, f_tile], F32, tag="sbp")
            nc.gpsimd.dma_start(sbp_t[:], sbp_r[t])

            # ---- smape halves: q = |pred - tgt| * 1/(pred + tgt) ----
            t1 = tmp.tile([P, f_tile], F32, tag="t1")
            nc.gpsimd.tensor_sub(t1[:], dbp_t[:], d_t[:])
            den1 = tmp.tile([P, f_tile], F32, tag="den1")
            nc.vector.tensor_add(den1[:], dbp_t[:], d_t[:])
            t2 = tmp.tile([P, f_tile], F32, tag="t2")
            nc.vector.tensor_sub(t2[:], sbp_t[:], s_t[:])
            den2 = tmp.tile([P, f_tile], F32, tag="den2")
            nc.gpsimd.tensor_add(den2[:], sbp_t[:], s_t[:])

            r1 = tmp.tile([P, f_tile], F32, tag="r1")
            nc.vector.reciprocal_approx_fast(out=r1[:], in_=den1[:])
            r2 = tmp.tile([P, f_tile], F32, tag="r2")
            nc.vector.reciprocal_approx_fast(out=r2[:], in_=den2[:])

            q1 = tmp.tile([P, f_tile], F32, tag="q1")
            nc.vector.scalar_tensor_tensor(
                q1[:], t1[:], 0.0, r1[:], op0=ALU.abs_max, op1=ALU.mult,
                accum_out=acc[:, base + 0 : base + 1],
            )
            q2 = tmp.tile([P, f_tile], F32, tag="q2")
            nc.vector.scalar_tensor_tensor(
                q2[:], t2[:], 0.0, r2[:], op0=ALU.abs_max, op1=ALU.mult,
                accum_out=acc[:, base + 1 : base + 2],
            )
            pe = tmp.tile([P, f_tile], F32, tag="pe")
            nc.vector.tensor_add(pe[:], q1[:], q2[:])

            # ---- mask margin tiles ----
            sd80 = tmp.tile([P, f_tile], F32, tag="sd80")
            nc.vector.tensor_scalar(sd80[:], d_t[:], 80.0, None, op0=ALU.subtract)
            d120 = tmp.tile([P, f_tile], F32, tag="d120")
            nc.gpsimd.tensor_scalar(d120[:], d_t[:], 120.0, None, op0=ALU.subtract)
            u1 = tmp.tile([P, f_tile], F32, tag="u1")
            nc.vector.tensor_scalar(u1[:], s_t[:], 135.0, 0.0, op0=ALU.subtract, op1=ALU.abs_max)
            u2 = tmp.tile([P, f_tile], F32, tag="u2")
            nc.gpsimd.tensor_scalar(u2[:], d_t[:], 85.0, 0.0, op0=ALU.subtract, op1=ALU.abs_max)

            m1 = tmp.tile([P, f_tile], F32, tag="m1")
            nc.gpsimd.scalar_tensor_tensor(
                m1[:], s_t[:], 120.0, sd80[:], op0=ALU.subtract, op1=ALU.max
            )
            m2 = tmp.tile([P, f_tile], F32, tag="m2")
            nc.vector.scalar_tensor_tensor(
                m2[:], s_t[:], 130.0, sd80[:], op0=ALU.subtract, op1=ALU.max
            )
            m3 = tmp.tile([P, f_tile], F32, tag="m3")
            nc.gpsimd.tensor_tensor(m3[:], u1[:], u2[:], op=ALU.min)
            m4 = tmp.tile([P, f_tile], F32, tag="m4")
            nc.vector.scalar_tensor_tensor(
                m4[:], s_t[:], 180.0, d120[:], op0=ALU.subtract, op1=ALU.max
            )

            # ---- masked sums (indicator * pe, fused accumulate) ----
            for ci, (m, thr, op) in enumerate(
                [
                    (m1, 0.0, ALU.is_lt),
                    (m2, 0.0, ALU.is_lt),
                    (m3, 5.0, ALU.is_lt),
                    (m4, 0.0, ALU.is_gt),
                ]
            ):
                o = scr.tile([P, f_tile], F32, tag="scr")
                eng = nc.vector if ci % 2 == 0 else nc.gpsimd
                eng.scalar_tensor_tensor(
                    o[:], m[:], thr, pe[:], op0=op, op1=ALU.mult,
                    accum_out=acc[:, base + 2 + ci : base + 3 + ci],
                )

            # ---- mask counts (indicator, fused accumulate) ----
            for ci, (m, thr, op) in enumerate(
                [
                    (m1, 0.0, ALU.is_lt),
                    (m2, 0.0, ALU.is_lt),
                    (m3, 5.0, ALU.is_lt),
                    (m4, 0.0, ALU.is_gt),
                ]
            ):
                o = scr.tile([P, f_tile], F32, tag="scr")
                eng = nc.gpsimd if ci % 2 == 0 else nc.vector
                eng.scalar_tensor_tensor(
                    o[:], m[:], thr, m[:], op0=op, op1=ALU.bypass,
                    accum_out=acc[:, base + 6 + ci : base + 7 + ci],
                )

        nc.gpsimd.dma_start(out_e[:], acc[:])

    return _split_excess_waits(nc) if split else nc


def _build_nc_v2(per_n=PER, f_tile=1024, split=True):
    """v2r: compare-path indicators (bf16) + TensorE diagonal-matmul for the
    masked sums and counts. pe is stored in 130-column blocks (128 data cols +
    a ones column + pad) so one matmul per (mask, block) yields both the
    masked-sum diagonal and the count column. Reciprocal + |.| with fused
    sum accumulation run on the Scalar engine."""
    n_tiles = per_n // (P * f_tile)
    n_blk = f_tile // P
    blkw = 130  # 128 data + 1 ones + 1 pad (4B alignment for bf16 2x mode)
    assert n_tiles * P * f_tile == per_n

    BF16 = mybir.dt.bfloat16
    nc = bass.Bass()
    s_e = nc.declare_dram_parameter("s", [per_n], F32, isOutput=False)
    d_e = nc.declare_dram_parameter("d", [per_n], F32, isOutput=False)
    dbp_e = nc.declare_dram_parameter("dbp", [per_n], F32, isOutput=False)
    sbp_e = nc.declare_dram_parameter("sbp", [per_n], F32, isOutput=False)
    out_q = nc.declare_dram_parameter("out", [P, 2 * n_tiles], F32, isOutput=True)
    out_d = nc.declare_dram_parameter("outd", [4, P, 129], F32, isOutput=True)

    s_r = s_e.rearrange("(t p f) -> t p f", p=P, f=f_tile)
    d_r = d_e.rearrange("(t p f) -> t p f", p=P, f=f_tile)
    dbp_r = dbp_e.rearrange("(t p f) -> t p f", p=P, f=f_tile)
    sbp_r = sbp_e.rearrange("(t p f) -> t p f", p=P, f=f_tile)

    with ExitStack() as ctx:
        tc = ctx.enter_context(tile.TileContext(nc))
        inp = ctx.enter_context(tc.tile_pool(name="inp", bufs=3))
        tmp = ctx.enter_context(tc.tile_pool(name="tmp", bufs=2))
        accp = ctx.enter_context(tc.tile_pool(name="acc", bufs=1))
        psum = ctx.enter_context(tc.tile_pool(name="psum", bufs=1, space="PSUM"))

        diag = [
            psum.tile([P, blkw], F32, tag=f"diag{i}", name=f"diag{i}")
            for i in range(4)
        ]
        acc = accp.tile([P, 2 * n_tiles], F32)
        bias_t = accp.tile([P, 3], F32)
        nc.gpsimd.memset(bias_t[:, 0:1], -135.0)
        nc.gpsimd.memset(bias_t[:, 1:2], -85.0)
        nc.gpsimd.memset(bias_t[:, 2:3], -120.0)

        def act_recip(out_ap, in_ap):
            nc.scalar.add_instruction(
                mybir.InstActivation(
                    name=nc.get_next_instruction_name(),
                    func=ACTF.Reciprocal,
                    ins=[
                        nc.scalar.lower_ap(in_ap),
                        mybir.ImmediateValue(dtype=F32, value=0.0),
                        mybir.ImmediateValue(dtype=F32, value=1.0),
                        mybir.ImmediateValue(dtype=F32, value=0.0),
                    ],
                    outs=[nc.scalar.lower_ap(out_ap)],
                )
            )

        for t in range(n_tiles):
            s_t = inp.tile([P, f_tile], F32, tag="s")
            nc.sync.dma_start(s_t[:], s_r[t])
            d_t = inp.tile([P, f_tile], F32, tag="d")
            nc.sync.dma_start(d_t[:], d_r[t])
            dbp_t = inp.tile([P, f_tile], F32, tag="dbp")
            nc.sync.dma_start(dbp_t[:], dbp_r[t])
            sbp_t = inp.tile([P, f_tile], F32, tag="sbp")
            nc.sync.dma_start(sbp_t[:], sbp_r[t])

            # ---- smape halves: q = |(pred - tgt) * recip(pred + tgt)| ----
            t1 = tmp.tile([P, f_tile], F32, tag="t1")
            nc.gpsimd.tensor_sub(t1[:], dbp_t[:], d_t[:])
            den1 = tmp.tile([P, f_tile], F32, tag="den1")
            nc.vector.tensor_add(den1[:], dbp_t[:], d_t[:])
            t2 = tmp.tile([P, f_tile], F32, tag="t2")
            nc.gpsimd.tensor_sub(t2[:], sbp_t[:], s_t[:])
            den2 = tmp.tile([P, f_tile], F32, tag="den2")
            nc.gpsimd.tensor_add(den2[:], sbp_t[:], s_t[:])

            r1 = tmp.tile([P, f_tile], F32, tag="r1")
            act_recip(r1[:], den1[:])
            r2 = tmp.tile([P, f_tile], F32, tag="r2")
            act_recip(r2[:], den2[:])

            w1 = tmp.tile([P, f_tile], F32, tag="w1")
            nc.gpsimd.tensor_mul(w1[:], t1[:], r1[:])
            w2 = tmp.tile([P, f_tile], F32, tag="w2")
            nc.vector.tensor_mul(w2[:], t2[:], r2[:])
            # q1 in 130-col blocks: cols k*130..k*130+127 data, k*130+128 ones
            aq1 = tmp.tile([P, n_blk * blkw], BF16, tag="aq1")
            aq1_3 = aq1[:].rearrange("p (b w) -> p b w", w=blkw)
            nc.vector.memset(aq1_3[:, :, 128:130], 1.0)
            nc.scalar.activation(
                aq1_3[:, :, 0:128], w1[:].rearrange("p (b w) -> p b w", w=P),
                ACTF.Abs, accum_out=acc[:, 2 * t : 2 * t + 1]
            )
            aq2 = tmp.tile([P, f_tile], BF16, tag="aq2")
            nc.scalar.activation(
                aq2[:], w2[:], ACTF.Abs, accum_out=acc[:, 2 * t + 1 : 2 * t + 2]
            )

            # ---- indicators (bf16, sign-exact) ----
            c1 = tmp.tile([P, f_tile], BF16, tag="c1")
            nc.vector.tensor_scalar(c1[:], s_t[:], 120.0, None, op0=ALU.is_lt)
            c2 = tmp.tile([P, f_tile], BF16, tag="c2")
            nc.vector.tensor_scalar(c2[:], s_t[:], 130.0, None, op0=ALU.is_lt)
            g1 = tmp.tile([P, f_tile], BF16, tag="g1")
            nc.vector.tensor_scalar(g1[:], d_t[:], 80.0, None, op0=ALU.is_lt)
            u1 = tmp.tile([P, f_tile], F32, tag="u1")
            nc.scalar.activation(u1[:], s_t[:], ACTF.Abs, bias=bias_t[:, 0:1])
            u2 = tmp.tile([P, f_tile], F32, tag="u2")
            nc.scalar.activation(u2[:], d_t[:], ACTF.Abs, bias=bias_t[:, 1:2])
            c4 = tmp.tile([P, f_tile], BF16, tag="c4")
            nc.vector.tensor_scalar(c4[:], s_t[:], 180.0, None, op0=ALU.is_gt)
            g3 = tmp.tile([P, f_tile], BF16, tag="g3")
            nc.scalar.activation(g3[:], d_t[:], ACTF.Sign, bias=bias_t[:, 2:3])

            p1m = tmp.tile([P, f_tile], BF16, tag="p1m")
            nc.gpsimd.tensor_mul(p1m[:], c1[:], g1[:])
            p2m = tmp.tile([P, f_tile], BF16, tag="p2m")
            nc.vector.tensor_mul(p2m[:], c2[:], g1[:])
            m3 = tmp.tile([P, f_tile], F32, tag="m3")
            nc.vector.tensor_tensor(m3[:], u1[:], u2[:], op=ALU.min)
            h1m = tmp.tile([P, f_tile], BF16, tag="h1m")
            nc.vector.tensor_scalar(h1m[:], m3[:], 5.0, None, op0=ALU.is_lt)
            crm = tmp.tile([P, f_tile], BF16, tag="crm")
            nc.vector.tensor_tensor(crm[:], c4[:], g3[:], op=ALU.max)

            for mi, mk in enumerate([p1m, p2m, h1m, crm]):
                for blk in range(n_blk):
                    first = t == 0 and blk == 0
                    last = t == n_tiles - 1 and blk == n_blk - 1
                    lo = blk * P
                    nc.tensor.matmul(
                        diag[mi][:, 0:129],
                        mk[:, lo : lo + P],
                        aq1[:, blk * blkw : blk * blkw + 129],
                        start=first,
                        stop=False,
                    )
                    nc.tensor.matmul(
                        diag[mi][:, 0:128],
                        mk[:, lo : lo + P],
                        aq2[:, lo : lo + P],
                        start=False,
                        stop=last,
                    )

        stage_d = accp.tile([P, 4 * 129], F32)
        for i in range(4):
            nc.vector.tensor_copy(stage_d[:, i * 129 : (i + 1) * 129], diag[i][:, 0:129])
            nc.sync.dma_start(out_d[i], stage_d[:, i * 129 : (i + 1) * 129])
        nc.sync.dma_start(out_q[:], acc[:])

    return _split_excess_waits(nc) if split else nc


VERSION = 2


def _get_nc(per_n=PER, f_tile=None):
    if f_tile is None:
        f_tile = 1024 if VERSION == 2 else F_TILE
    key = (VERSION, per_n, f_tile)
    if key not in _NC_CACHE:
        builder = _build_nc_v2 if VERSION == 2 else _build_nc
        _NC_CACHE[key] = builder(per_n, f_tile)
    return _NC_CACHE[key]


def _finalize(vec10, batch_n):
    """Host-side: combine the 10 global partial sums into the loss (f64)."""
    sq1, sq2, s_p1, s_p2, s_h1, s_cr, c_p1, c_p2, c_h1, c_cr = [
        float(x) for x in vec10
    ]
    s_tot = sq1 + sq2
    # factor 2 from smape definition
    S = [
        2.0 * s_p1,                       # normal
        2.0 * (s_p2 - s_p1),              # elevated
        2.0 * s_h1,                       # hyper1
        2.0 * (s_tot - s_p2 - s_h1),      # hyper2
        2.0 * s_cr,                       # crisis
    ]
    C = [
        c_p1,
        c_p2 - c_p1,
        c_h1,
        batch_n - c_p2 - c_h1,
        c_cr,
    ]
    rst = 0.0
    m_rst = 0.0
    mask_cnt = 0
    for s_m, cnt in zip(S, C):
        w = np.sqrt(np.log(batch_n / max(cnt, 1.0)))
        if cnt > 0:
            m_rst = (m_rst + s_m * w) / cnt / 2.0
            rst = rst + m_rst
            mask_cnt += 1
    if mask_cnt == 0:
        return rst / 5.0
    return rst / mask_cnt


def host_partials(s, d, dbp, sbp):
    """Numpy replica of the device partials (for testing)."""
    s = s.astype(np.float64)
    d = d.astype(np.float64)
    dbp = dbp.astype(np.float64)
    sbp = sbp.astype(np.float64)
    q1 = np.abs(dbp - d) / (dbp + d)
    q2 = np.abs(sbp - s) / (sbp + s)
    pe = q1 + q2
    m1 = np.maximum(s - 120, d - 80)
    m2 = np.maximum(s - 130, d - 80)
    m3 = np.minimum(np.abs(s - 135), np.abs(d - 85))
    m4 = np.maximum(s - 180, d - 120)
    return np.array(
        [
            q1.sum(),
            q2.sum(),
            pe[m1 < 0].sum(),
            pe[m2 < 0].sum(),
            pe[m3 < 5].sum(),
            pe[m4 > 0].sum(),
            (m1 < 0).sum(),
            (m2 < 0).sum(),
            (m3 < 5).sum(),
            (m4 > 0).sum(),
        ]
    )


def kernel(**inputs):
    s = np.ascontiguousarray(np.asarray(inputs["s"], dtype=np.float32).reshape(-1))
    d = np.ascontiguousarray(np.asarray(inputs["d"], dtype=np.float32).reshape(-1))
    dbp = np.ascontiguousarray(
        np.asarray(inputs["dbp_pred"], dtype=np.float32).reshape(-1)
    )
    sbp = np.ascontiguousarray(
        np.asarray(inputs["sbp_pred"], dtype=np.float32).reshape(-1)
    )
    batch_n = s.shape[0]
    assert batch_n == B, f"expected {B}, got {batch_n}"

    nc = _get_nc()
    in_maps = []
    for c in range(NCORES):
        sl = slice(c * PER, (c + 1) * PER)
        in_maps.append({"s": s[sl], "d": d[sl], "dbp": dbp[sl], "sbp": sbp[sl]})

    res = run_bass_kernel_spmd(nc, in_maps, list(range(NCORES)), trace=TRACE)
    LAST_RESULT["exec_time_ns"] = res.exec_time_ns
    LAST_RESULT["raw"] = res

    tot = np.zeros(N_COLS, np.float64)
    for r in res.results:
        if VERSION == 2:
            q = np.asarray(r["out"], np.float64)  # [P, 2*n_tiles]
            diag = np.asarray(r["outd"], np.float64)  # [4, P, 130]
            tot[0] += q[:, 0::2].sum()
            tot[1] += q[:, 1::2].sum()
            for i in range(4):
                tot[2 + i] += np.trace(diag[i, :, 0:128])
                tot[6 + i] += diag[i, :, 128].sum()
        else:
            o = np.asarray(r["out"], np.float64).reshape(P, N_TILES, N_COLS)
            tot += o.sum(axis=(0, 1))
    loss = _finalize(tot, float(batch_n))
    return np.float32(loss)
